# revision 15
# baseline (speedup 1.0000x reference)
"""MoE layer (dense all-expert routing) Trainium2 Bass kernel.

Problem: x[4,2048,1024] f32, gate_w[1024,8], gate_b[8], expert_w[8,1024,1024].
  gate = softmax(x @ gate_w + gate_b)                  # [B,S,E]
  out  = einsum('bse,bseo->bso', gate, einsum('bsi,eio->bseo', x, expert_w))

Sharding: data-parallel over tokens. 8192 tokens split into 8 shards of 1024;
each core computes its shard against all 8 experts (weights replicated).
No collectives; host concatenates shard outputs.

Per-core kernel (tuned against perfetto traces; 250.5us -> ~242us):
  - all matmuls bf16 with f32 PSUM accumulation (rel err ~2.7e-3)
  - head: all in-flight DMAs round-robin and complete together near the
    end of the aggregate transfer (+~2.3us trigger/completion latency
    each), so the input load is an arrival-paced LADDER: a minimal first
    wave (gate weights, x_k0 split across both HWDGE rings, w0's k0
    slice), then one-step-lookahead waves chained on earlier chunks'
    completions. Chains are pure Tile deps: a 1-element gpsimd copy from
    the trigger chunk into the next chunk's tile makes the next
    dma_start a WAW-ordered later writer.
  - ~50 N=128 warm-up matmuls bridge sequencer-start -> first-data and
    hold the HAM clock gate at 8/8 (a >1.7us idle gap re-throttles to
    1.2GHz for ~3.4us; x_k0 arrival jitters 12.4-14.3us so the bridge
    overshoots)
  - gate: logits accumulated per k-section (8 single-matmul PSUM groups
    into a scratch bank — start+stop on every matmul, immune to the
    whole-bank has_written clear — then one DVE op folds the scratch
    into an SBUF accumulator), interleaved with expert 0's k-sections so
    each section needs only the k-th x chunk. Bias applied as a
    host-precomputed exp(b) columnwise multiply after exp; softmax =
    exp, fused mul+sum, reciprocal, scale.
  - main loop: per (n-half, expert) one 1MiB weight DMA on the gpsimd
    SWDGE ring (its gentle ~90GB/s drip doesn't contend with the PE's
    SBUF reads; only head-critical W1 rides HWDGE, chained on x_k45),
    64 matmuls, then per m one fused DVE op: acc = (psum*g[:,e]) + acc
  - final (n,e) block runs every m-group as two N=256 PSUM half-groups
    (the very last m-group as four N=128 quarters) so each output DMA
    launches well before the last matmul and the critical tail shrinks
  - steady state measured at the stream floor: 215-216ns per N=512
    matmul, zero PE gaps; remaining costs are the ~8us sequencer boot,
    ~4.4us head DMA latency, ~4.3us tail (output DMA completion +
    epilogue barrier), and a ~160ns stall every 10.79us (HBM refresh-
    like artifact)
"""

import numpy as np
import ml_dtypes
from contextlib import ExitStack

import concourse.bacc as bacc
import concourse.bass as bass
import concourse.mybir as mybir
import concourse.tile as tile

BF16 = mybir.dt.bfloat16
F32 = mybir.dt.float32
FP8 = mybir.dt.float8e4  # TRN FP8_EXP4 == ml_dtypes.float8_e4m3 (IEEE, max +-240)

P = 128  # partitions


def build_moe_nc(T=1024, D=1024, O=1024, E=8, NO=512, w_bufs=2, acc_bufs=16):
    """Build the per-core Bass program.

    T: tokens per core, D: d_in, O: d_out, E: experts, NO: d_out tile (<=512).
    """
    KT = D // P   # k tiles (contraction)
    MT = T // P   # token tiles
    NT = O // NO  # d_out tiles

    nc = bacc.Bacc("TRN2", target_bir_lowering=False, debug=False)
    xT_d = nc.dram_tensor("xT", [D, T], BF16, kind="ExternalInput")
    w_d = nc.dram_tensor("w", [E, D, O], BF16, kind="ExternalInput")
    # fp8 copies of the first two k-tiles for the DoubleRow (2x) passes:
    # x8T[p, j, t] = e4m3(x[t, j*128+p]);  w8[e, p, j, o] = e4m3(64*w[e, j*128+p, o])
    x8T_d = nc.dram_tensor("x8T", [P, 2, T], FP8, kind="ExternalInput")
    w8_d = nc.dram_tensor("w8", [E, P, 2, O], FP8, kind="ExternalInput")
    # gwt[p, k*E+e] = gate_w[k*128+p, e]  (host pre-tiled, contiguous DMA)
    gwt_d = nc.dram_tensor("gwt", [P, KT * E], BF16, kind="ExternalInput")
    # expb[p, e] = exp(gate_b[e]) replicated across partitions
    expb_d = nc.dram_tensor("expb", [P, E], F32, kind="ExternalInput")
    # flat tile-major output: each DMA writes one CONTIGUOUS HBM block
    # (vs 128 rows strided 4KB apart) — fewer descriptors, cheaper
    # trigger gen + WAW completion on the kernel's critical tail.
    # span for tile (m, col0, ncols): [m*P*O + col0*P, + P*ncols)
    out_d = nc.dram_tensor("out", [T * O], F32, kind="ExternalOutput")

    with tile.TileContext(nc) as tc:
        with ExitStack() as ctx:
            singles = ctx.enter_context(tc.tile_pool(name="singles", bufs=1))
            wpool = ctx.enter_context(tc.tile_pool(name="w", bufs=w_bufs))
            w8pool = ctx.enter_context(tc.tile_pool(name="w8", bufs=w_bufs))
            accp = ctx.enter_context(tc.tile_pool(name="acc", bufs=acc_bufs))
            gpool = ctx.enter_context(tc.tile_pool(name="gate", bufs=1))
            ps = ctx.enter_context(tc.tile_pool(name="ps", bufs=7, space="PSUM"))

            # ---- resident loads -------------------------------------------
            # HAM warm-up: short N=128 dummy matmuls on a memset tile keep
            # the PE busy from sequencer-start until the first x chunk
            # lands (~4us: DMA trigger + ~2.3us completion latency +
            # transfer), so the clock gate is at 8/8 when real work starts
            # and the first real matmuls run at full clock.
            warm = singles.tile([P, P], BF16, tag="warm")
            nc.vector.memset(warm, 0.0)
            psw = ps.tile([P, P], F32, tag="psg", bufs=1, name="psw")
            # enough to cover worst-case x_k0 arrival (~10us with the
            # 3-queue head below). Too SHORT is expensive: a >1us PE gap
            # here resets the HAM busy window and the first ~5us of real
            # matmuls run at 1.2GHz (measured +4us). Too long only costs
            # ~0.1us/dummy.
            NWARM = 30
            for j in range(NWARM):
                nc.tensor.matmul(
                    psw, lhsT=warm, rhs=warm,
                    start=(j == 0), stop=(j == NWARM - 1),
                )

            # Head loads, all on the two HWDGE rings, staged as an
            # arrival-paced LADDER. All concurrently in-flight DMAs
            # round-robin at packet granularity and complete together near
            # the end of the aggregate transfer (measured: every head DMA
            # of an all-at-once plan lands at 20-27us), so instead each
            # wave's trigger is chained on the previous x chunk's
            # completion. The chain is expressed as a pure Tile dep: a
            # 1-element DVE copy from the previous chunk into the next
            # chunk's tile forces the next dma_start (a WAW-ordered later
            # writer) to wait for the copy, which waits for the data.
            wt0 = wpool.tile([P, KT, NO], BF16, tag="w", name="wt0")
            w0_src = w_d[0, :, 0:NO].rearrange("(k p) o -> p k o", p=P)
            xparts = {}
            xtiles = {}

            def chain_src(chain_on):
                # chain_on: chunk index, or (chunk index, token col)
                kc, col = chain_on if isinstance(chain_on, tuple) else (
                    chain_on, 0
                )
                return xtiles[kc][0:1, 0:1, col:col + 1]

            def load_x(kc, nk, eng, chain_on=None):
                t = singles.tile(
                    [P, nk, T], BF16, tag=f"xT{kc}", name=f"xc{kc}"
                )
                if chain_on is not None:
                    # chain copies ride the otherwise-idle gpsimd engine —
                    # on the in-order DVE they'd block the gate's
                    # accumulate ops behind later rungs of the ladder
                    nc.gpsimd.tensor_copy(
                        t[0:1, 0:1, 0:1], chain_src(chain_on)
                    )
                eng.dma_start(
                    out=t,
                    in_=xT_d[kc * P:(kc + nk) * P, :].rearrange(
                        "(k p) t -> p k t", p=P
                    ),
                )
                xtiles[kc] = t
                for i in range(nk):
                    xparts[kc + i] = (t, i)

            def load_w0(kc, nk, eng, chain_on=None):
                if chain_on is not None:
                    nc.gpsimd.tensor_copy(
                        wt0[0:1, kc:kc + 1, 0:1], chain_src(chain_on)
                    )
                eng.dma_start(
                    out=wt0[:, kc:kc + nk, :], in_=w0_src[:, kc:kc + nk, :]
                )

            # wave 1 (in flight immediately, ~420KB): consts + x_k0 split
            # across BOTH rings + w0's k0 slice — the minimal set for the
            # gate-k0 + e0-k0 sections, so the PE's first real work starts
            # as early as the DMA path allows. Later waves are chained
            # with one-step LOOKAHEAD (rung i+1 fires on rung i-1's
            # completion) so the ~2-3us per-DMA trigger+completion latency
            # pipelines away while keeping at most ~1.5MB in flight (full
            # concurrency degrades to everything-lands-at-the-end
            # round-robin).
            # x_k0's halves trigger FIRST on each ring — each DIRECT2D
            # descriptor-gen costs ~0.65us of sequencer time, so consts
            # queued ahead of x would delay the whole pipeline start
            # the first wave rides THREE parallel queues: x_k0 half A on
            # the sync ring, gate weights + x_k0 half B on the scalar
            # ring, w0_k0/k1 on the SWDGE ring (its ~1us first-byte is
            # fine — they're consumed after the gate-k0 section). Rungs
            # are emitted in FIRE-TIME order per queue (each queue is
            # strict FIFO; a rung queued behind a later-firing one waits)
            TH = T // 2
            xc0 = singles.tile([P, 1, T], BF16, tag="xT0", name="xc0")
            nc.sync.dma_start(out=xc0[:, 0, 0:TH], in_=xT_d[0:P, 0:TH])
            gw_t = singles.tile([P, KT, E], BF16, tag="gw")
            nc.scalar.dma_start(
                out=gw_t, in_=gwt_d[:, :].rearrange("p (k e) -> p k e", e=E)
            )
            nc.scalar.dma_start(out=xc0[:, 0, TH:T], in_=xT_d[0:P, TH:T])
            xtiles[0] = xc0
            xparts[0] = (xc0, 0)
            expb_sb = singles.tile([P, E], F32, tag="expb")
            nc.scalar.dma_start(out=expb_sb, in_=expb_d[:, :])
            load_w0(0, 1, nc.gpsimd)
            load_w0(1, 1, nc.gpsimd, chain_on=0)         # on x_k0 half A
            # rung chains: key (tile, token col) identifies the half/chunk
            # whose completion triggers the rung
            load_x(1, 1, nc.scalar, chain_on=0)          # on x_k0 half A
            load_x(2, 2, nc.sync, chain_on=0)            # on x_k0 half A
            load_w0(2, 2, nc.scalar, chain_on=(0, TH))   # on x_k0 half B
            load_w0(4, 2, nc.scalar, chain_on=1)         # on x_k1
            load_x(4, 2, nc.sync, chain_on=2)            # on x_k23
            load_w0(6, 2, nc.scalar, chain_on=2)         # on x_k23
            load_x(6, 2, nc.sync, chain_on=4)            # on x_k45
            # fp8 copy of x k-tiles 0-1 for the DoubleRow passes; first
            # consumed by block (n0, e1) ~14us after the stream starts, so
            # chain it deep (and BEFORE the e1 weight chain copies, which
            # fire at the same x_k45 arrival, to keep gpsimd FIFO order)
            x8_sb = singles.tile([P, 2, T], FP8, tag="x8", name="x8")
            nc.gpsimd.tensor_copy(x8_sb[0:1, 0:1, 0:1], chain_src(4))
            nc.sync.dma_start(out=x8_sb, in_=x8T_d[:, :, :])

            def xT(k):
                t, i = xparts[k]
                return t[:, i, :]

            # ---- gate ------------------------------------------------------
            # Logits are accumulated per k-section so each section only
            # needs the k-th x chunk: 8 single-matmul PSUM groups (start+
            # stop on every matmul — immune to the whole-bank has_written
            # clear that start=True performs, so no cross-group corruption)
            # into a scratch bank, then one DVE op folds the scratch into
            # an SBUF accumulator. Interleaved with expert 0's k-sections.
            gacc = gpool.tile([P, MT * E], F32, tag="gacc", name="gacc")

            def emit_gate_k(k):
                scr = ps.tile(
                    [P, MT * E], F32, tag="psg", bufs=1, name=f"gsc{k}"
                )
                for m in range(MT):
                    nc.tensor.matmul(
                        scr[:, m * E:(m + 1) * E],
                        lhsT=xT(k)[:, m * P:(m + 1) * P],
                        rhs=gw_t[:, k, :],
                        start=True,
                        stop=True,
                    )
                if k == 0:
                    nc.vector.tensor_copy(gacc, scr)
                else:
                    nc.vector.scalar_tensor_tensor(
                        out=gacc,
                        in0=scr,
                        scalar=1.0,
                        in1=gacc,
                        op0=mybir.AluOpType.mult,
                        op1=mybir.AluOpType.add,
                    )

            g_sb = [None] * MT

            def emit_gate():
                for m in range(MT):
                    # softmax with bias folded in multiplicatively:
                    # g = exp(l)*exp(b) / sum_e exp(l)*exp(b)
                    p_t = gpool.tile([P, E], F32, tag=f"p{m}", name=f"p{m}")
                    # exp(logits); |logits| <~ 3 so no max-subtraction needed
                    nc.scalar.activation(
                        p_t, gacc[:, m * E:(m + 1) * E],
                        mybir.ActivationFunctionType.Exp,
                    )
                    q_t = gpool.tile([P, E], F32, tag=f"q{m}", name=f"q{m}")
                    s_t = gpool.tile([P, 1], F32, tag=f"s{m}", name=f"s{m}")
                    # q = p * expb (and s = sum_e q in the same op)
                    nc.vector.scalar_tensor_tensor(
                        out=q_t,
                        in0=p_t,
                        scalar=1.0,
                        in1=expb_sb,
                        op0=mybir.AluOpType.mult,
                        op1=mybir.AluOpType.mult,
                        accum_out=s_t,
                    )
                    rs_t = gpool.tile([P, 1], F32, tag=f"rs{m}", name=f"rs{m}")
                    nc.vector.reciprocal(rs_t, s_t)
                    g_t = gpool.tile([P, E], F32, tag=f"g{m}", name=f"g{m}")
                    nc.vector.tensor_scalar_mul(g_t, q_t, rs_t)
                    g_sb[m] = g_t

            # ---- main: all-expert GEMM + fused gate combine ---------------
            for n in range(NT):
                acc = [None] * MT
                for e in range(E):
                    # one 1MiB DMA per (n, e): all k-tiles of this d_out
                    # slice. (n==0, e==0) was loaded k-granular at the head.
                    is_final = (n == NT - 1 and e == E - 1)
                    if n == 0 and e == 0:
                        wt = wt0
                    else:
                        wt = wpool.tile([P, KT, NO], BF16, tag="w")
                        if n == 0 and e == 1:
                            # expert 1 is head-critical: chain it on
                            # x_k45's arrival (so it doesn't steal ladder
                            # bandwidth) and use HWDGE (SWDGE would land
                            # it ~10us/MB later). All later experts ride
                            # the SWDGE drip: its gentle ~90GB/s delivery
                            # doesn't contend with the PE's SBUF reads,
                            # while HWDGE's ~300GB/s bursts stretch every
                            # in-flight matmul (measured +48ns/MM).
                            nc.gpsimd.tensor_copy(
                                wt[0:1, 2:3, 0:1], xtiles[4][0:1, 0:1, 0:1]
                            )
                            eng = nc.scalar
                        else:
                            eng = nc.gpsimd
                        if is_final:
                            # final block stays full-bf16 (accuracy head-
                            # room is spent on the 14 middle blocks)
                            eng.dma_start(
                                out=wt,
                                in_=w_d[e, :, n * NO:(n + 1) * NO].rearrange(
                                    "(k p) o -> p k o", p=P
                                ),
                            )
                        else:
                            # middle block: k-tiles 0-1 ride the fp8
                            # DoubleRow pass, so only k2-7 in bf16
                            wt8 = w8pool.tile([P, 2, NO], FP8, tag="w8")
                            if n == 0 and e == 1:
                                nc.gpsimd.tensor_copy(
                                    wt8[0:1, 0:1, 0:1],
                                    xtiles[4][0:1, 0:1, 0:1],
                                )
                            eng.dma_start(
                                out=wt8,
                                in_=w8_d[e, :, :, n * NO:(n + 1) * NO],
                            )
                            eng.dma_start(
                                out=wt[:, 2:, :],
                                in_=w_d[
                                    e, 2 * P:, n * NO:(n + 1) * NO
                                ].rearrange("(k p) o -> p k o", p=P),
                            )
                    # Expert 0 (head, DMA-paced): k-outer so the PE can
                    # consume each arriving k chunk across all m groups.
                    # Experts 1+: m-outer — each PSUM group is 8 consecutive
                    # matmuls, slots cycle fast, and the per-m combine +
                    # output DMA spread across the stream.
                    if n == 0 and e == 0:
                        psy_l = [None] * MT
                        for k in range(KT):
                            # gate section first: 8 tiny matmuls whose
                            # inputs (gw + chunk k) are already resident —
                            # extra ready work that absorbs DMA jitter
                            emit_gate_k(k)
                            for m in range(MT - 1):
                                if k == 0:
                                    psy_l[m] = ps.tile(
                                        [P, NO], F32, tag="ps", name=f"psk{m}"
                                    )
                                nc.tensor.matmul(
                                    psy_l[m],
                                    lhsT=xT(k)[:, m * P:(m + 1) * P],
                                    rhs=wt[:, k, :],
                                    start=(k == 0),
                                    stop=(k == KT - 1),
                                )
                        psy_l[MT - 1] = ps.tile(
                            [P, NO], F32, tag="ps", name="psk_last"
                        )
                        for k in range(KT):
                            nc.tensor.matmul(
                                psy_l[MT - 1],
                                lhsT=xT(k)[:, (MT - 1) * P:MT * P],
                                rhs=wt[:, k, :],
                                start=(k == 0),
                                stop=(k == KT - 1),
                            )
                        # copies first (no gate dep — frees the PSUM slots
                        # even though the gate hasn't run), then the gate,
                        # then the g0 scale-muls which need it
                        for m in range(MT):
                            acc[m] = accp.tile(
                                [P, NO], F32, tag="acc", name=f"acc{m}"
                            )
                            nc.vector.tensor_copy(acc[m], psy_l[m])
                        emit_gate()
                        for m in range(MT):
                            nc.vector.tensor_scalar_mul(
                                acc[m], acc[m], g_sb[m][:, 0:1]
                            )
                        continue
                    if n == NT - 1 and e == E - 1:
                        # final block: every m-group as two N=256 PSUM
                        # half-groups so each half's combine + 128KB output
                        # DMA launches as soon as that half completes —
                        # the output stream drains alongside the matmuls
                        # instead of piling up after the last one
                        for m in range(MT):
                            # the very last m-group goes quarter-granular
                            # (N=128): the final combine shrinks 483->~300ns
                            # and the final DMA transfer halves — both sit
                            # on the kernel's critical tail
                            nsplit = 4 if m == MT - 1 else 2
                            NH = NO // nsplit
                            for h in range(nsplit):
                                psy_h = ps.tile(
                                    [P, NH], F32, tag="ps",
                                    name=f"psyh{m}_{h}"
                                )
                                for k in range(KT):
                                    nc.tensor.matmul(
                                        psy_h,
                                        lhsT=xT(k)[:, m * P:(m + 1) * P],
                                        rhs=wt[:, k, h * NH:(h + 1) * NH],
                                        start=(k == 0),
                                        stop=(k == KT - 1),
                                    )
                                nc.vector.scalar_tensor_tensor(
                                    out=acc[m][:, h * NH:(h + 1) * NH],
                                    in0=psy_h,
                                    scalar=g_sb[m][:, e:e + 1],
                                    in1=acc[m][:, h * NH:(h + 1) * NH],
                                    op0=mybir.AluOpType.mult,
                                    op1=mybir.AluOpType.add,
                                )
                                eng = nc.scalar if h % 2 == 0 else nc.sync
                                off = m * P * O + (n * NO + h * NH) * P
                                eng.dma_start(
                                    out=out_d[off:off + P * NH].rearrange(
                                        "(p c) -> p c", p=P
                                    ),
                                    in_=acc[m][:, h * NH:(h + 1) * NH],
                                )
                        continue
                    # k-tiles 0-1 ride one fp8 DoubleRow matmul per m
                    # (2 weights/cell -> K=256 per pass); k2-7 accumulate
                    # on top in bf16. NOTE: grouping consecutive DR
                    # matmuls (to pull LDWEIGHTS ahead) wedges the PE
                    # (NRT_EXEC_UNIT_UNRECOVERABLE) — keep them
                    # interleaved one per m-group.
                    for m in range(MT):
                        psy = ps.tile([P, NO], F32, tag="ps", name=f"psy{m}")
                        nc.tensor.matmul(
                            psy,
                            lhsT=x8_sb[:, :, m * P:(m + 1) * P],
                            rhs=wt8[:, :, :],
                            start=True,
                            stop=False,
                            perf_mode=mybir.MatmulPerfMode.DoubleRow,
                        )
                        for k in range(2, KT):
                            nc.tensor.matmul(
                                psy,
                                lhsT=xT(k)[:, m * P:(m + 1) * P],
                                rhs=wt[:, k, :],
                                start=False,
                                stop=(k == KT - 1),
                            )
                        if e == 0:
                            # init acc with an unscaled copy (no gate dep —
                            # frees the PSUM slot even if the gate is still
                            # running), then fold g0 in as a separate op
                            acc[m] = accp.tile(
                                [P, NO], F32, tag="acc", name=f"acc{m}"
                            )
                            nc.vector.tensor_copy(acc[m], psy)
                            nc.vector.tensor_scalar_mul(
                                acc[m], acc[m], g_sb[m][:, 0:1]
                            )
                        else:
                            nc.vector.scalar_tensor_tensor(
                                out=acc[m],
                                in0=psy,
                                scalar=g_sb[m][:, e:e + 1],
                                in1=acc[m],
                                op0=mybir.AluOpType.mult,
                                op1=mybir.AluOpType.add,
                            )
                        if e == E - 1:
                            eng = nc.sync if m % 2 == 0 else nc.scalar
                            off = m * P * O + n * NO * P
                            eng.dma_start(
                                out=out_d[off:off + P * NO].rearrange(
                                    "(p c) -> p c", p=P
                                ),
                                in_=acc[m],
                            )
    nc.compile()
    return nc


# ---------------------------------------------------------------------------
# Host wrapper: full inputs -> shard -> run SPMD on 8 cores -> gather
# ---------------------------------------------------------------------------

N_CORES = 8
_B, _S, _DIN, _DOUT, _E = 4, 2048, 1024, 1024, 8


def _host_gwt(gate_w):
    """[D, E] -> [128, KT*E] with gwt[p, k*E+e] = gate_w[k*128+p, e]."""
    D, E = gate_w.shape
    kt = D // P
    return np.ascontiguousarray(
        gate_w.reshape(kt, P, E).transpose(1, 0, 2).reshape(P, kt * E)
    )


LAST_RESULTS = None  # BassKernelResults of the most recent run (for profiling)


def kernel(x, gate_w, gate_b, expert_w, _trace=False):
    global LAST_RESULTS
    from concourse.bass_utils import run_bass_kernel_spmd

    x = np.asarray(x)
    tokens = x.reshape(-1, _DIN)  # [8192, 1024]
    n_tok = tokens.shape[0]
    tpc = n_tok // N_CORES  # tokens per core

    # ALL weights are shipped scaled by 64 (exact power-of-2, so the bf16
    # path is bit-identical after the host-side /64 of the output) to put
    # the fp8 copies of k-tiles 0-1 in e4m3's normal range.
    w64 = np.asarray(expert_w, dtype=np.float32) * np.float32(64.0)
    w_bf = w64.astype(ml_dtypes.bfloat16)
    # w8[e, p, j, o] = e4m3(64*w[e, j*128+p, o]) for k-tiles j=0,1
    w8 = np.ascontiguousarray(
        w64[:, : 2 * P, :]
        .reshape(_E, 2, P, _DOUT)
        .transpose(0, 2, 1, 3)
    ).astype(ml_dtypes.float8_e4m3)
    gwt_bf = _host_gwt(np.asarray(gate_w)).astype(ml_dtypes.bfloat16)
    expb = np.broadcast_to(
        np.exp(np.asarray(gate_b, dtype=np.float64)).astype(np.float32),
        (P, _E),
    ).copy()

    in_maps = []
    for c in range(N_CORES):
        shard = tokens[c * tpc:(c + 1) * tpc]  # [1024, 1024]
        xT32 = np.ascontiguousarray(shard.T)   # [1024(D), 1024(T)] f32
        xT = xT32.astype(ml_dtypes.bfloat16)
        # x8T[p, j, t] = e4m3(x[t, j*128+p]) for k-tiles j=0,1
        x8T = np.ascontiguousarray(
            xT32[: 2 * P].reshape(2, P, tpc).transpose(1, 0, 2)
        ).astype(ml_dtypes.float8_e4m3)
        in_maps.append(
            {"xT": xT, "x8T": x8T, "w": w_bf, "w8": w8,
             "gwt": gwt_bf, "expb": expb}
        )

    nc = build_moe_nc(T=tpc, D=_DIN, O=_DOUT, E=_E)
    res = run_bass_kernel_spmd(nc, in_maps, list(range(N_CORES)), trace=_trace)
    LAST_RESULTS = res
    # reassemble the flat tile-major output (must mirror the kernel's
    # tile map: n0 full-width per m; final block halves, m7 quarters)
    tile_map = [(m, 0, 512) for m in range(8)]
    for m in range(7):
        tile_map += [(m, 512, 256), (m, 768, 256)]
    tile_map += [(7, 512 + q * 128, 128) for q in range(4)]
    tpc_o = tpc * _DOUT
    parts = []
    for c in range(N_CORES):
        flat = np.asarray(res.results[c]["out"]).reshape(tpc_o)
        full_c = np.empty((tpc, _DOUT), dtype=np.float32)
        for (m, c0, ncol) in tile_map:
            off = m * P * _DOUT + c0 * P
            full_c[m * P:(m + 1) * P, c0:c0 + ncol] = flat[
                off:off + P * ncol
            ].reshape(P, ncol)
        parts.append(full_c)
    full = np.concatenate(parts, axis=0).astype(np.float32)
    full *= np.float32(1.0 / 64.0)  # undo the x64 weight scaling (exact)
    return full.reshape(_B, _S, _DOUT)



# revision 18
# speedup vs baseline: 1.0016x; 1.0016x over previous
"""MoE layer (dense all-expert routing) Trainium2 Bass kernel.

Problem: x[4,2048,1024] f32, gate_w[1024,8], gate_b[8], expert_w[8,1024,1024].
  gate = softmax(x @ gate_w + gate_b)                  # [B,S,E]
  out  = einsum('bse,bseo->bso', gate, einsum('bsi,eio->bseo', x, expert_w))

Sharding: data-parallel over tokens. 8192 tokens split into 8 shards of 1024;
each core computes its shard against all 8 experts (weights replicated).
No collectives; host concatenates shard outputs.

Per-core kernel (tuned against perfetto traces; 250.5us -> ~242us):
  - all matmuls bf16 with f32 PSUM accumulation (rel err ~2.7e-3)
  - head: all in-flight DMAs round-robin and complete together near the
    end of the aggregate transfer (+~2.3us trigger/completion latency
    each), so the input load is an arrival-paced LADDER: a minimal first
    wave (gate weights, x_k0 split across both HWDGE rings, w0's k0
    slice), then one-step-lookahead waves chained on earlier chunks'
    completions. Chains are pure Tile deps: a 1-element gpsimd copy from
    the trigger chunk into the next chunk's tile makes the next
    dma_start a WAW-ordered later writer.
  - ~50 N=128 warm-up matmuls bridge sequencer-start -> first-data and
    hold the HAM clock gate at 8/8 (a >1.7us idle gap re-throttles to
    1.2GHz for ~3.4us; x_k0 arrival jitters 12.4-14.3us so the bridge
    overshoots)
  - gate: logits accumulated per k-section (8 single-matmul PSUM groups
    into a scratch bank — start+stop on every matmul, immune to the
    whole-bank has_written clear — then one DVE op folds the scratch
    into an SBUF accumulator), interleaved with expert 0's k-sections so
    each section needs only the k-th x chunk. Bias applied as a
    host-precomputed exp(b) columnwise multiply after exp; softmax =
    exp, fused mul+sum, reciprocal, scale.
  - main loop: per (n-half, expert) one 1MiB weight DMA on the gpsimd
    SWDGE ring (its gentle ~90GB/s drip doesn't contend with the PE's
    SBUF reads; only head-critical W1 rides HWDGE, chained on x_k45),
    64 matmuls, then per m one fused DVE op: acc = (psum*g[:,e]) + acc
  - final (n,e) block runs every m-group as two N=256 PSUM half-groups
    (the very last m-group as four N=128 quarters) so each output DMA
    launches well before the last matmul and the critical tail shrinks
  - steady state measured at the stream floor: 215-216ns per N=512
    matmul, zero PE gaps; remaining costs are the ~8us sequencer boot,
    ~4.4us head DMA latency, ~4.3us tail (output DMA completion +
    epilogue barrier), and a ~160ns stall every 10.79us (HBM refresh-
    like artifact)
"""

import numpy as np
import ml_dtypes
from contextlib import ExitStack

import concourse.bacc as bacc
import concourse.bass as bass
import concourse.mybir as mybir
import concourse.tile as tile

BF16 = mybir.dt.bfloat16
F32 = mybir.dt.float32
FP8 = mybir.dt.float8e4  # TRN FP8_EXP4 == ml_dtypes.float8_e4m3 (IEEE, max +-240)

P = 128  # partitions


def build_moe_nc(T=1024, D=1024, O=1024, E=8, NO=512, w_bufs=2, acc_bufs=16):
    """Build the per-core Bass program.

    T: tokens per core, D: d_in, O: d_out, E: experts, NO: d_out tile (<=512).
    """
    KT = D // P   # k tiles (contraction)
    MT = T // P   # token tiles
    NT = O // NO  # d_out tiles

    nc = bacc.Bacc("TRN2", target_bir_lowering=False, debug=False)
    xT_d = nc.dram_tensor("xT", [D, T], BF16, kind="ExternalInput")
    w_d = nc.dram_tensor("w", [E, D, O], BF16, kind="ExternalInput")
    # fp8 copies of the first two k-tiles for the DoubleRow (2x) passes:
    # x8T[p, j, t] = e4m3(x[t, j*128+p]);  w8[e, p, j, o] = e4m3(64*w[e, j*128+p, o])
    x8T_d = nc.dram_tensor("x8T", [P, 2, T], FP8, kind="ExternalInput")
    w8_d = nc.dram_tensor("w8", [E, P, 2, O], FP8, kind="ExternalInput")
    # gwt[p, k*E+e] = gate_w[k*128+p, e]  (host pre-tiled, contiguous DMA)
    gwt_d = nc.dram_tensor("gwt", [P, KT * E], BF16, kind="ExternalInput")
    # expb[p, e] = exp(gate_b[e]) replicated across partitions
    expb_d = nc.dram_tensor("expb", [P, E], F32, kind="ExternalInput")
    # flat tile-major output: each DMA writes one CONTIGUOUS HBM block
    # (vs 128 rows strided 4KB apart) — fewer descriptors, cheaper
    # trigger gen + WAW completion on the kernel's critical tail.
    # span for tile (m, col0, ncols): [m*P*O + col0*P, + P*ncols)
    out_d = nc.dram_tensor("out", [T * O], F32, kind="ExternalOutput")

    with tile.TileContext(nc) as tc:
        with ExitStack() as ctx:
            singles = ctx.enter_context(tc.tile_pool(name="singles", bufs=1))
            wpool = ctx.enter_context(tc.tile_pool(name="w", bufs=w_bufs))
            w8pool = ctx.enter_context(tc.tile_pool(name="w8", bufs=w_bufs))
            accp = ctx.enter_context(tc.tile_pool(name="acc", bufs=acc_bufs))
            gpool = ctx.enter_context(tc.tile_pool(name="gate", bufs=1))
            ps = ctx.enter_context(tc.tile_pool(name="ps", bufs=7, space="PSUM"))

            # ---- resident loads -------------------------------------------
            # HAM warm-up: short N=128 dummy matmuls on a memset tile keep
            # the PE busy from sequencer-start until the first x chunk
            # lands (~4us: DMA trigger + ~2.3us completion latency +
            # transfer), so the clock gate is at 8/8 when real work starts
            # and the first real matmuls run at full clock.
            warm = singles.tile([P, P], BF16, tag="warm")
            nc.vector.memset(warm, 0.0)
            psw = ps.tile([P, P], F32, tag="psg", bufs=1, name="psw")
            # enough to cover worst-case x_k0 arrival (~10us with the
            # 3-queue head below). Too SHORT is expensive: a >1us PE gap
            # here resets the HAM busy window and the first ~5us of real
            # matmuls run at 1.2GHz (measured +4us). Too long only costs
            # ~0.1us/dummy.
            NWARM = 30
            for j in range(NWARM):
                nc.tensor.matmul(
                    psw, lhsT=warm, rhs=warm,
                    start=(j == 0), stop=(j == NWARM - 1),
                )

            # Head loads, all on the two HWDGE rings, staged as an
            # arrival-paced LADDER. All concurrently in-flight DMAs
            # round-robin at packet granularity and complete together near
            # the end of the aggregate transfer (measured: every head DMA
            # of an all-at-once plan lands at 20-27us), so instead each
            # wave's trigger is chained on the previous x chunk's
            # completion. The chain is expressed as a pure Tile dep: a
            # 1-element DVE copy from the previous chunk into the next
            # chunk's tile forces the next dma_start (a WAW-ordered later
            # writer) to wait for the copy, which waits for the data.
            wt0 = wpool.tile([P, KT, NO], BF16, tag="w", name="wt0")
            w0_src = w_d[0, :, 0:NO].rearrange("(k p) o -> p k o", p=P)
            xparts = {}
            xtiles = {}

            def chain_src(chain_on):
                # chain_on: chunk index, or (chunk index, token col)
                kc, col = chain_on if isinstance(chain_on, tuple) else (
                    chain_on, 0
                )
                return xtiles[kc][0:1, 0:1, col:col + 1]

            def load_x(kc, nk, eng, chain_on=None):
                t = singles.tile(
                    [P, nk, T], BF16, tag=f"xT{kc}", name=f"xc{kc}"
                )
                if chain_on is not None:
                    # chain copies ride the otherwise-idle gpsimd engine —
                    # on the in-order DVE they'd block the gate's
                    # accumulate ops behind later rungs of the ladder
                    nc.gpsimd.tensor_copy(
                        t[0:1, 0:1, 0:1], chain_src(chain_on)
                    )
                eng.dma_start(
                    out=t,
                    in_=xT_d[kc * P:(kc + nk) * P, :].rearrange(
                        "(k p) t -> p k t", p=P
                    ),
                )
                xtiles[kc] = t
                for i in range(nk):
                    xparts[kc + i] = (t, i)

            def load_w0(kc, nk, eng, chain_on=None):
                if chain_on is not None:
                    nc.gpsimd.tensor_copy(
                        wt0[0:1, kc:kc + 1, 0:1], chain_src(chain_on)
                    )
                eng.dma_start(
                    out=wt0[:, kc:kc + nk, :], in_=w0_src[:, kc:kc + nk, :]
                )

            # wave 1 (in flight immediately, ~420KB): consts + x_k0 split
            # across BOTH rings + w0's k0 slice — the minimal set for the
            # gate-k0 + e0-k0 sections, so the PE's first real work starts
            # as early as the DMA path allows. Later waves are chained
            # with one-step LOOKAHEAD (rung i+1 fires on rung i-1's
            # completion) so the ~2-3us per-DMA trigger+completion latency
            # pipelines away while keeping at most ~1.5MB in flight (full
            # concurrency degrades to everything-lands-at-the-end
            # round-robin).
            # x_k0's halves trigger FIRST on each ring — each DIRECT2D
            # descriptor-gen costs ~0.65us of sequencer time, so consts
            # queued ahead of x would delay the whole pipeline start
            # the first wave rides THREE parallel queues: x_k0 half A on
            # the sync ring, gate weights + x_k0 half B on the scalar
            # ring, w0_k0/k1 on the SWDGE ring (its ~1us first-byte is
            # fine — they're consumed after the gate-k0 section). Rungs
            # are emitted in FIRE-TIME order per queue (each queue is
            # strict FIFO; a rung queued behind a later-firing one waits)
            TH = T // 2
            xc0 = singles.tile([P, 1, T], BF16, tag="xT0", name="xc0")
            nc.sync.dma_start(out=xc0[:, 0, 0:TH], in_=xT_d[0:P, 0:TH])
            gw_t = singles.tile([P, KT, E], BF16, tag="gw")
            nc.scalar.dma_start(
                out=gw_t, in_=gwt_d[:, :].rearrange("p (k e) -> p k e", e=E)
            )
            nc.scalar.dma_start(out=xc0[:, 0, TH:T], in_=xT_d[0:P, TH:T])
            xtiles[0] = xc0
            xparts[0] = (xc0, 0)
            expb_sb = singles.tile([P, E], F32, tag="expb")
            nc.scalar.dma_start(out=expb_sb, in_=expb_d[:, :])
            load_w0(0, 1, nc.gpsimd)
            # rung chains: key (tile, token col) identifies the half/chunk
            # whose completion triggers the rung
            load_x(1, 1, nc.scalar, chain_on=0)          # on x_k0 half A
            load_w0(1, 1, nc.sync, chain_on=0)
            load_x(2, 2, nc.sync, chain_on=(0, TH))      # on x_k0 half B
            load_w0(2, 2, nc.scalar, chain_on=(0, TH))
            load_x(4, 2, nc.sync, chain_on=(0, TH))      # on x_k0 half B
            load_w0(4, 2, nc.scalar, chain_on=1)         # on x_k1
            load_x(6, 2, nc.sync, chain_on=1)            # on x_k1
            load_w0(6, 2, nc.scalar, chain_on=1)         # on x_k1
            # fp8 copy of x k-tiles 0-1 for the DoubleRow passes; first
            # consumed by block (n0, e1) ~16us after the stream starts, so
            # chain it deepest (x_k67) — and BEFORE the e1 weight chain
            # copies, which fire on the same arrival (gpsimd FIFO order)
            x8_sb = singles.tile([P, 2, T], FP8, tag="x8", name="x8")
            nc.gpsimd.tensor_copy(x8_sb[0:1, 0:1, 0:1], chain_src(6))
            nc.sync.dma_start(out=x8_sb, in_=x8T_d[:, :, :])

            def xT(k):
                t, i = xparts[k]
                return t[:, i, :]

            # ---- gate ------------------------------------------------------
            # Logits are accumulated per k-section so each section only
            # needs the k-th x chunk: 8 single-matmul PSUM groups (start+
            # stop on every matmul — immune to the whole-bank has_written
            # clear that start=True performs, so no cross-group corruption)
            # into a scratch bank, then one DVE op folds the scratch into
            # an SBUF accumulator. Interleaved with expert 0's k-sections.
            gacc = gpool.tile([P, MT * E], F32, tag="gacc", name="gacc")

            def emit_gate_k(k):
                scr = ps.tile(
                    [P, MT * E], F32, tag="psg", bufs=1, name=f"gsc{k}"
                )
                for m in range(MT):
                    nc.tensor.matmul(
                        scr[:, m * E:(m + 1) * E],
                        lhsT=xT(k)[:, m * P:(m + 1) * P],
                        rhs=gw_t[:, k, :],
                        start=True,
                        stop=True,
                    )
                if k == 0:
                    nc.vector.tensor_copy(gacc, scr)
                else:
                    nc.vector.scalar_tensor_tensor(
                        out=gacc,
                        in0=scr,
                        scalar=1.0,
                        in1=gacc,
                        op0=mybir.AluOpType.mult,
                        op1=mybir.AluOpType.add,
                    )

            g_sb = [None] * MT

            def emit_gate():
                for m in range(MT):
                    # softmax with bias folded in multiplicatively:
                    # g = exp(l)*exp(b) / sum_e exp(l)*exp(b)
                    p_t = gpool.tile([P, E], F32, tag=f"p{m}", name=f"p{m}")
                    # exp(logits); |logits| <~ 3 so no max-subtraction needed
                    nc.scalar.activation(
                        p_t, gacc[:, m * E:(m + 1) * E],
                        mybir.ActivationFunctionType.Exp,
                    )
                    q_t = gpool.tile([P, E], F32, tag=f"q{m}", name=f"q{m}")
                    s_t = gpool.tile([P, 1], F32, tag=f"s{m}", name=f"s{m}")
                    # q = p * expb (and s = sum_e q in the same op)
                    nc.vector.scalar_tensor_tensor(
                        out=q_t,
                        in0=p_t,
                        scalar=1.0,
                        in1=expb_sb,
                        op0=mybir.AluOpType.mult,
                        op1=mybir.AluOpType.mult,
                        accum_out=s_t,
                    )
                    rs_t = gpool.tile([P, 1], F32, tag=f"rs{m}", name=f"rs{m}")
                    nc.vector.reciprocal(rs_t, s_t)
                    g_t = gpool.tile([P, E], F32, tag=f"g{m}", name=f"g{m}")
                    nc.vector.tensor_scalar_mul(g_t, q_t, rs_t)
                    g_sb[m] = g_t

            # ---- main: all-expert GEMM + fused gate combine ---------------
            for n in range(NT):
                acc = [None] * MT
                for e in range(E):
                    # one 1MiB DMA per (n, e): all k-tiles of this d_out
                    # slice. (n==0, e==0) was loaded k-granular at the head.
                    is_final = (n == NT - 1 and e == E - 1)
                    if n == 0 and e == 0:
                        wt = wt0
                    else:
                        wt = wpool.tile([P, KT, NO], BF16, tag="w")
                        if n == 0 and e == 1:
                            # expert 1 is head-critical: chain it on
                            # x_k45's arrival (so it doesn't steal ladder
                            # bandwidth) and use HWDGE (SWDGE would land
                            # it ~10us/MB later). All later experts ride
                            # the SWDGE drip: its gentle ~90GB/s delivery
                            # doesn't contend with the PE's SBUF reads,
                            # while HWDGE's ~300GB/s bursts stretch every
                            # in-flight matmul (measured +48ns/MM).
                            nc.gpsimd.tensor_copy(
                                wt[0:1, 2:3, 0:1], xtiles[6][0:1, 0:1, 0:1]
                            )
                            eng = nc.scalar
                        else:
                            eng = nc.gpsimd
                        if is_final:
                            # final block stays full-bf16 (accuracy head-
                            # room is spent on the 14 middle blocks)
                            eng.dma_start(
                                out=wt,
                                in_=w_d[e, :, n * NO:(n + 1) * NO].rearrange(
                                    "(k p) o -> p k o", p=P
                                ),
                            )
                        else:
                            # middle block: k-tiles 0-1 ride the fp8
                            # DoubleRow pass, so only k2-7 in bf16
                            wt8 = w8pool.tile([P, 2, NO], FP8, tag="w8")
                            if n == 0 and e == 1:
                                nc.gpsimd.tensor_copy(
                                    wt8[0:1, 0:1, 0:1],
                                    xtiles[6][0:1, 0:1, 0:1],
                                )
                            eng.dma_start(
                                out=wt8,
                                in_=w8_d[e, :, :, n * NO:(n + 1) * NO],
                            )
                            eng.dma_start(
                                out=wt[:, 2:, :],
                                in_=w_d[
                                    e, 2 * P:, n * NO:(n + 1) * NO
                                ].rearrange("(k p) o -> p k o", p=P),
                            )
                    # Expert 0 (head, DMA-paced): k-outer so the PE can
                    # consume each arriving k chunk across all m groups.
                    # Experts 1+: m-outer — each PSUM group is 8 consecutive
                    # matmuls, slots cycle fast, and the per-m combine +
                    # output DMA spread across the stream.
                    if n == 0 and e == 0:
                        psy_l = [None] * MT
                        for k in range(KT):
                            # gate section first: 8 tiny matmuls whose
                            # inputs (gw + chunk k) are already resident —
                            # extra ready work that absorbs DMA jitter
                            emit_gate_k(k)
                            for m in range(MT - 1):
                                if k == 0:
                                    psy_l[m] = ps.tile(
                                        [P, NO], F32, tag="ps", name=f"psk{m}"
                                    )
                                nc.tensor.matmul(
                                    psy_l[m],
                                    lhsT=xT(k)[:, m * P:(m + 1) * P],
                                    rhs=wt[:, k, :],
                                    start=(k == 0),
                                    stop=(k == KT - 1),
                                )
                        psy_l[MT - 1] = ps.tile(
                            [P, NO], F32, tag="ps", name="psk_last"
                        )
                        for k in range(KT):
                            nc.tensor.matmul(
                                psy_l[MT - 1],
                                lhsT=xT(k)[:, (MT - 1) * P:MT * P],
                                rhs=wt[:, k, :],
                                start=(k == 0),
                                stop=(k == KT - 1),
                            )
                        # copies first (no gate dep — frees the PSUM slots
                        # even though the gate hasn't run), then the gate,
                        # then the g0 scale-muls which need it
                        for m in range(MT):
                            acc[m] = accp.tile(
                                [P, NO], F32, tag="acc", name=f"acc{m}"
                            )
                            nc.vector.tensor_copy(acc[m], psy_l[m])
                        emit_gate()
                        for m in range(MT):
                            nc.vector.tensor_scalar_mul(
                                acc[m], acc[m], g_sb[m][:, 0:1]
                            )
                        continue
                    if n == NT - 1 and e == E - 1:
                        # final block: every m-group as two N=256 PSUM
                        # half-groups so each half's combine + 128KB output
                        # DMA launches as soon as that half completes —
                        # the output stream drains alongside the matmuls
                        # instead of piling up after the last one
                        for m in range(MT):
                            # the very last m-group goes quarter-granular
                            # (N=128): the final combine shrinks 483->~300ns
                            # and the final DMA transfer halves — both sit
                            # on the kernel's critical tail
                            nsplit = 4 if m == MT - 1 else 2
                            NH = NO // nsplit
                            for h in range(nsplit):
                                psy_h = ps.tile(
                                    [P, NH], F32, tag="ps",
                                    name=f"psyh{m}_{h}"
                                )
                                for k in range(KT):
                                    nc.tensor.matmul(
                                        psy_h,
                                        lhsT=xT(k)[:, m * P:(m + 1) * P],
                                        rhs=wt[:, k, h * NH:(h + 1) * NH],
                                        start=(k == 0),
                                        stop=(k == KT - 1),
                                    )
                                nc.vector.scalar_tensor_tensor(
                                    out=acc[m][:, h * NH:(h + 1) * NH],
                                    in0=psy_h,
                                    scalar=g_sb[m][:, e:e + 1],
                                    in1=acc[m][:, h * NH:(h + 1) * NH],
                                    op0=mybir.AluOpType.mult,
                                    op1=mybir.AluOpType.add,
                                )
                                eng = nc.scalar if h % 2 == 0 else nc.sync
                                off = m * P * O + (n * NO + h * NH) * P
                                eng.dma_start(
                                    out=out_d[off:off + P * NH].rearrange(
                                        "(p c) -> p c", p=P
                                    ),
                                    in_=acc[m][:, h * NH:(h + 1) * NH],
                                )
                        continue
                    # k-tiles 0-1 ride one fp8 DoubleRow matmul per m
                    # (2 weights/cell -> K=256 per pass); k2-7 accumulate
                    # on top in bf16. NOTE: grouping consecutive DR
                    # matmuls (to pull LDWEIGHTS ahead) wedges the PE
                    # (NRT_EXEC_UNIT_UNRECOVERABLE) — keep them
                    # interleaved one per m-group.
                    for m in range(MT):
                        psy = ps.tile([P, NO], F32, tag="ps", name=f"psy{m}")
                        nc.tensor.matmul(
                            psy,
                            lhsT=x8_sb[:, :, m * P:(m + 1) * P],
                            rhs=wt8[:, :, :],
                            start=True,
                            stop=False,
                            perf_mode=mybir.MatmulPerfMode.DoubleRow,
                        )
                        for k in range(2, KT):
                            nc.tensor.matmul(
                                psy,
                                lhsT=xT(k)[:, m * P:(m + 1) * P],
                                rhs=wt[:, k, :],
                                start=False,
                                stop=(k == KT - 1),
                            )
                        if e == 0:
                            # init acc with an unscaled copy (no gate dep —
                            # frees the PSUM slot even if the gate is still
                            # running), then fold g0 in as a separate op
                            acc[m] = accp.tile(
                                [P, NO], F32, tag="acc", name=f"acc{m}"
                            )
                            nc.vector.tensor_copy(acc[m], psy)
                            nc.vector.tensor_scalar_mul(
                                acc[m], acc[m], g_sb[m][:, 0:1]
                            )
                        else:
                            nc.vector.scalar_tensor_tensor(
                                out=acc[m],
                                in0=psy,
                                scalar=g_sb[m][:, e:e + 1],
                                in1=acc[m],
                                op0=mybir.AluOpType.mult,
                                op1=mybir.AluOpType.add,
                            )
                        if e == E - 1:
                            eng = nc.sync if m % 2 == 0 else nc.scalar
                            off = m * P * O + n * NO * P
                            eng.dma_start(
                                out=out_d[off:off + P * NO].rearrange(
                                    "(p c) -> p c", p=P
                                ),
                                in_=acc[m],
                            )
    nc.compile()
    return nc


# ---------------------------------------------------------------------------
# Host wrapper: full inputs -> shard -> run SPMD on 8 cores -> gather
# ---------------------------------------------------------------------------

N_CORES = 8
_B, _S, _DIN, _DOUT, _E = 4, 2048, 1024, 1024, 8


def _host_gwt(gate_w):
    """[D, E] -> [128, KT*E] with gwt[p, k*E+e] = gate_w[k*128+p, e]."""
    D, E = gate_w.shape
    kt = D // P
    return np.ascontiguousarray(
        gate_w.reshape(kt, P, E).transpose(1, 0, 2).reshape(P, kt * E)
    )


LAST_RESULTS = None  # BassKernelResults of the most recent run (for profiling)


def kernel(x, gate_w, gate_b, expert_w, _trace=False):
    global LAST_RESULTS
    from concourse.bass_utils import run_bass_kernel_spmd

    x = np.asarray(x)
    tokens = x.reshape(-1, _DIN)  # [8192, 1024]
    n_tok = tokens.shape[0]
    tpc = n_tok // N_CORES  # tokens per core

    # ALL weights are shipped scaled by 64 (exact power-of-2, so the bf16
    # path is bit-identical after the host-side /64 of the output) to put
    # the fp8 copies of k-tiles 0-1 in e4m3's normal range.
    w64 = np.asarray(expert_w, dtype=np.float32) * np.float32(64.0)
    w_bf = w64.astype(ml_dtypes.bfloat16)
    # w8[e, p, j, o] = e4m3(64*w[e, j*128+p, o]) for k-tiles j=0,1
    w8 = np.ascontiguousarray(
        w64[:, : 2 * P, :]
        .reshape(_E, 2, P, _DOUT)
        .transpose(0, 2, 1, 3)
    ).astype(ml_dtypes.float8_e4m3)
    gwt_bf = _host_gwt(np.asarray(gate_w)).astype(ml_dtypes.bfloat16)
    expb = np.broadcast_to(
        np.exp(np.asarray(gate_b, dtype=np.float64)).astype(np.float32),
        (P, _E),
    ).copy()

    in_maps = []
    for c in range(N_CORES):
        shard = tokens[c * tpc:(c + 1) * tpc]  # [1024, 1024]
        xT32 = np.ascontiguousarray(shard.T)   # [1024(D), 1024(T)] f32
        xT = xT32.astype(ml_dtypes.bfloat16)
        # x8T[p, j, t] = e4m3(x[t, j*128+p]) for k-tiles j=0,1
        x8T = np.ascontiguousarray(
            xT32[: 2 * P].reshape(2, P, tpc).transpose(1, 0, 2)
        ).astype(ml_dtypes.float8_e4m3)
        in_maps.append(
            {"xT": xT, "x8T": x8T, "w": w_bf, "w8": w8,
             "gwt": gwt_bf, "expb": expb}
        )

    nc = build_moe_nc(T=tpc, D=_DIN, O=_DOUT, E=_E)
    res = run_bass_kernel_spmd(nc, in_maps, list(range(N_CORES)), trace=_trace)
    LAST_RESULTS = res
    # reassemble the flat tile-major output (must mirror the kernel's
    # tile map: n0 full-width per m; final block halves, m7 quarters)
    tile_map = [(m, 0, 512) for m in range(8)]
    for m in range(7):
        tile_map += [(m, 512, 256), (m, 768, 256)]
    tile_map += [(7, 512 + q * 128, 128) for q in range(4)]
    tpc_o = tpc * _DOUT
    parts = []
    for c in range(N_CORES):
        flat = np.asarray(res.results[c]["out"]).reshape(tpc_o)
        full_c = np.empty((tpc, _DOUT), dtype=np.float32)
        for (m, c0, ncol) in tile_map:
            off = m * P * _DOUT + c0 * P
            full_c[m * P:(m + 1) * P, c0:c0 + ncol] = flat[
                off:off + P * ncol
            ].reshape(P, ncol)
        parts.append(full_c)
    full = np.concatenate(parts, axis=0).astype(np.float32)
    full *= np.float32(1.0 / 64.0)  # undo the x64 weight scaling (exact)
    return full.reshape(_B, _S, _DOUT)



# revision 21
# speedup vs baseline: 1.0060x; 1.0043x over previous
"""MoE layer (dense all-expert routing) Trainium2 Bass kernel.

Problem: x[4,2048,1024] f32, gate_w[1024,8], gate_b[8], expert_w[8,1024,1024].
  gate = softmax(x @ gate_w + gate_b)                  # [B,S,E]
  out  = einsum('bse,bseo->bso', gate, einsum('bsi,eio->bseo', x, expert_w))

Sharding: data-parallel over tokens. 8192 tokens split into 8 shards of 1024;
each core computes its shard against all 8 experts (weights replicated).
No collectives; host concatenates shard outputs.

Per-core kernel (tuned against perfetto traces; 250.5us -> ~242us):
  - all matmuls bf16 with f32 PSUM accumulation (rel err ~2.7e-3)
  - head: all in-flight DMAs round-robin and complete together near the
    end of the aggregate transfer (+~2.3us trigger/completion latency
    each), so the input load is an arrival-paced LADDER: a minimal first
    wave (gate weights, x_k0 split across both HWDGE rings, w0's k0
    slice), then one-step-lookahead waves chained on earlier chunks'
    completions. Chains are pure Tile deps: a 1-element gpsimd copy from
    the trigger chunk into the next chunk's tile makes the next
    dma_start a WAW-ordered later writer.
  - ~50 N=128 warm-up matmuls bridge sequencer-start -> first-data and
    hold the HAM clock gate at 8/8 (a >1.7us idle gap re-throttles to
    1.2GHz for ~3.4us; x_k0 arrival jitters 12.4-14.3us so the bridge
    overshoots)
  - gate: logits accumulated per k-section (8 single-matmul PSUM groups
    into a scratch bank — start+stop on every matmul, immune to the
    whole-bank has_written clear — then one DVE op folds the scratch
    into an SBUF accumulator), interleaved with expert 0's k-sections so
    each section needs only the k-th x chunk. Bias applied as a
    host-precomputed exp(b) columnwise multiply after exp; softmax =
    exp, fused mul+sum, reciprocal, scale.
  - main loop: per (n-half, expert) one 1MiB weight DMA on the gpsimd
    SWDGE ring (its gentle ~90GB/s drip doesn't contend with the PE's
    SBUF reads; only head-critical W1 rides HWDGE, chained on x_k45),
    64 matmuls, then per m one fused DVE op: acc = (psum*g[:,e]) + acc
  - final (n,e) block runs every m-group as two N=256 PSUM half-groups
    (the very last m-group as four N=128 quarters) so each output DMA
    launches well before the last matmul and the critical tail shrinks
  - steady state measured at the stream floor: 215-216ns per N=512
    matmul, zero PE gaps; remaining costs are the ~8us sequencer boot,
    ~4.4us head DMA latency, ~4.3us tail (output DMA completion +
    epilogue barrier), and a ~160ns stall every 10.79us (HBM refresh-
    like artifact)
"""

import numpy as np
import ml_dtypes
from contextlib import ExitStack

import concourse.bacc as bacc
import concourse.bass as bass
import concourse.mybir as mybir
import concourse.tile as tile

BF16 = mybir.dt.bfloat16
F32 = mybir.dt.float32
FP8 = mybir.dt.float8e4  # TRN FP8_EXP4 == ml_dtypes.float8_e4m3 (IEEE, max +-240)

P = 128  # partitions


def build_moe_nc(T=1024, D=1024, O=1024, E=8, NO=512, w_bufs=2, acc_bufs=16):
    """Build the per-core Bass program.

    T: tokens per core, D: d_in, O: d_out, E: experts, NO: d_out tile (<=512).
    """
    KT = D // P   # k tiles (contraction)
    MT = T // P   # token tiles
    NT = O // NO  # d_out tiles

    nc = bacc.Bacc("TRN2", target_bir_lowering=False, debug=False)
    xT_d = nc.dram_tensor("xT", [D, T], BF16, kind="ExternalInput")
    w_d = nc.dram_tensor("w", [E, D, O], BF16, kind="ExternalInput")
    # fp8 copies of the first two k-tiles for the DoubleRow (2x) passes:
    # x8T[p, j, t] = e4m3(x[t, j*128+p]);  w8[e, p, j, o] = e4m3(64*w[e, j*128+p, o])
    x8T_d = nc.dram_tensor("x8T", [P, 2, T], FP8, kind="ExternalInput")
    w8_d = nc.dram_tensor("w8", [E, P, 2, O], FP8, kind="ExternalInput")
    # gwt[p, k*E+e] = gate_w[k*128+p, e]  (host pre-tiled, contiguous DMA)
    gwt_d = nc.dram_tensor("gwt", [P, KT * E], BF16, kind="ExternalInput")
    # expb[p, e] = exp(gate_b[e]) replicated across partitions
    expb_d = nc.dram_tensor("expb", [P, E], F32, kind="ExternalInput")
    # flat tile-major output: each DMA writes one CONTIGUOUS HBM block
    # (vs 128 rows strided 4KB apart) — fewer descriptors, cheaper
    # trigger gen + WAW completion on the kernel's critical tail.
    # span for tile (m, col0, ncols): [m*P*O + col0*P, + P*ncols)
    out_d = nc.dram_tensor("out", [T * O], F32, kind="ExternalOutput")

    with tile.TileContext(nc) as tc:
        with ExitStack() as ctx:
            singles = ctx.enter_context(tc.tile_pool(name="singles", bufs=1))
            wpool = ctx.enter_context(tc.tile_pool(name="w", bufs=w_bufs))
            w8pool = ctx.enter_context(tc.tile_pool(name="w8", bufs=w_bufs))
            accp = ctx.enter_context(tc.tile_pool(name="acc", bufs=acc_bufs))
            gpool = ctx.enter_context(tc.tile_pool(name="gate", bufs=1))
            ps = ctx.enter_context(tc.tile_pool(name="ps", bufs=7, space="PSUM"))

            # ---- resident loads -------------------------------------------
            # HAM warm-up: short N=128 dummy matmuls on a memset tile keep
            # the PE busy from sequencer-start until the first x chunk
            # lands (~4us: DMA trigger + ~2.3us completion latency +
            # transfer), so the clock gate is at 8/8 when real work starts
            # and the first real matmuls run at full clock.
            warm = singles.tile([P, P], BF16, tag="warm")
            nc.vector.memset(warm, 0.0)
            psw = ps.tile([P, P], F32, tag="psg", bufs=1, name="psw")
            # sized so the warmups bridge seamlessly to the first real
            # section (~11.2us): ends ~11.25 at 107ns/dummy from ~6.97.
            # Too SHORT is expensive: a >1us PE gap resets the HAM busy
            # window and the first ~5us of real matmuls run at 1.2GHz
            # (measured +2.5us); too long only costs ~0.1us/dummy. A
            # continuous bridge also means HAM unthrottles at ~10.4us so
            # the real stream runs warm from its first matmul.
            NWARM = 40
            for j in range(NWARM):
                nc.tensor.matmul(
                    psw, lhsT=warm, rhs=warm,
                    start=(j == 0), stop=(j == NWARM - 1),
                )

            # Head loads, all on the two HWDGE rings, staged as an
            # arrival-paced LADDER. All concurrently in-flight DMAs
            # round-robin at packet granularity and complete together near
            # the end of the aggregate transfer (measured: every head DMA
            # of an all-at-once plan lands at 20-27us), so instead each
            # wave's trigger is chained on the previous x chunk's
            # completion. The chain is expressed as a pure Tile dep: a
            # 1-element DVE copy from the previous chunk into the next
            # chunk's tile forces the next dma_start (a WAW-ordered later
            # writer) to wait for the copy, which waits for the data.
            wt0 = wpool.tile([P, KT, NO], BF16, tag="w", name="wt0")
            w0_src = w_d[0, :, 0:NO].rearrange("(k p) o -> p k o", p=P)
            xparts = {}
            xtiles = {}

            def chain_src(chain_on):
                # chain_on: chunk index, or (chunk index, token col)
                kc, col = chain_on if isinstance(chain_on, tuple) else (
                    chain_on, 0
                )
                return xtiles[kc][0:1, 0:1, col:col + 1]

            def load_x(kc, nk, eng, chain_on=None):
                t = singles.tile(
                    [P, nk, T], BF16, tag=f"xT{kc}", name=f"xc{kc}"
                )
                if chain_on is not None:
                    # chain copies ride the otherwise-idle gpsimd engine —
                    # on the in-order DVE they'd block the gate's
                    # accumulate ops behind later rungs of the ladder
                    nc.gpsimd.tensor_copy(
                        t[0:1, 0:1, 0:1], chain_src(chain_on)
                    )
                eng.dma_start(
                    out=t,
                    in_=xT_d[kc * P:(kc + nk) * P, :].rearrange(
                        "(k p) t -> p k t", p=P
                    ),
                )
                xtiles[kc] = t
                for i in range(nk):
                    xparts[kc + i] = (t, i)

            def load_w0(kc, nk, eng, chain_on=None):
                if chain_on is not None:
                    nc.gpsimd.tensor_copy(
                        wt0[0:1, kc:kc + 1, 0:1], chain_src(chain_on)
                    )
                eng.dma_start(
                    out=wt0[:, kc:kc + nk, :], in_=w0_src[:, kc:kc + nk, :]
                )

            # wave 1 (in flight immediately, ~420KB): consts + x_k0 split
            # across BOTH rings + w0's k0 slice — the minimal set for the
            # gate-k0 + e0-k0 sections, so the PE's first real work starts
            # as early as the DMA path allows. Later waves are chained
            # with one-step LOOKAHEAD (rung i+1 fires on rung i-1's
            # completion) so the ~2-3us per-DMA trigger+completion latency
            # pipelines away while keeping at most ~1.5MB in flight (full
            # concurrency degrades to everything-lands-at-the-end
            # round-robin).
            # x_k0's halves trigger FIRST on each ring — each DIRECT2D
            # descriptor-gen costs ~0.65us of sequencer time, so consts
            # queued ahead of x would delay the whole pipeline start
            # the first wave rides THREE parallel queues: x_k0 half A on
            # the sync ring, gate weights + x_k0 half B on the scalar
            # ring, w0_k0/k1 on the SWDGE ring (its ~1us first-byte is
            # fine — they're consumed after the gate-k0 section). Rungs
            # are emitted in FIRE-TIME order per queue (each queue is
            # strict FIFO; a rung queued behind a later-firing one waits)
            TH = T // 2
            xc0 = singles.tile([P, 1, T], BF16, tag="xT0", name="xc0")
            nc.sync.dma_start(out=xc0[:, 0, 0:TH], in_=xT_d[0:P, 0:TH])
            nc.scalar.dma_start(out=xc0[:, 0, TH:T], in_=xT_d[0:P, TH:T])
            xtiles[0] = xc0
            xparts[0] = (xc0, 0)
            gw_t = singles.tile([P, KT, E], BF16, tag="gw")
            nc.gpsimd.dma_start(
                out=gw_t, in_=gwt_d[:, :].rearrange("p (k e) -> p k e", e=E)
            )
            expb_sb = singles.tile([P, E], F32, tag="expb")
            nc.scalar.dma_start(out=expb_sb, in_=expb_d[:, :])
            load_w0(0, 1, nc.gpsimd)
            # rung chains: key (tile, token col) identifies the half/chunk
            # whose completion triggers the rung
            load_x(1, 1, nc.scalar, chain_on=0)          # on x_k0 half A
            load_w0(1, 1, nc.sync, chain_on=0)
            load_x(2, 2, nc.sync, chain_on=(0, TH))      # on x_k0 half B
            load_w0(2, 2, nc.scalar, chain_on=(0, TH))
            load_x(4, 2, nc.sync, chain_on=(0, TH))      # on x_k0 half B
            load_w0(4, 2, nc.scalar, chain_on=1)         # on x_k1
            load_x(6, 2, nc.sync, chain_on=1)            # on x_k1
            # w0_k67 isn't consumed until ~11us after the first section —
            # chain it deepest so its 256KB doesn't dilute x's bandwidth
            # share in the early delivery window
            load_w0(6, 2, nc.scalar, chain_on=4)         # on x_k45
            # fp8 copy of x k-tiles 0-1 for the DoubleRow passes; first
            # consumed by block (n0, e1) ~16us after the stream starts, so
            # chain it deep — and BEFORE the e1 weight chain copies, which
            # fire on the same arrival (gpsimd FIFO order)
            x8_sb = singles.tile([P, 2, T], FP8, tag="x8", name="x8")
            nc.gpsimd.tensor_copy(x8_sb[0:1, 0:1, 0:1], chain_src(4))
            nc.sync.dma_start(out=x8_sb, in_=x8T_d[:, :, :])

            def xT(k):
                t, i = xparts[k]
                return t[:, i, :]

            # ---- gate ------------------------------------------------------
            # Logits are accumulated per k-section so each section only
            # needs the k-th x chunk: 8 single-matmul PSUM groups (start+
            # stop on every matmul — immune to the whole-bank has_written
            # clear that start=True performs, so no cross-group corruption)
            # into a scratch bank, then one DVE op folds the scratch into
            # an SBUF accumulator. Interleaved with expert 0's k-sections.
            gacc = gpool.tile([P, MT * E], F32, tag="gacc", name="gacc")

            def emit_gate_k(k):
                scr = ps.tile(
                    [P, MT * E], F32, tag="psg", bufs=1, name=f"gsc{k}"
                )
                for m in range(MT):
                    nc.tensor.matmul(
                        scr[:, m * E:(m + 1) * E],
                        lhsT=xT(k)[:, m * P:(m + 1) * P],
                        rhs=gw_t[:, k, :],
                        start=True,
                        stop=True,
                    )
                if k == 0:
                    nc.vector.tensor_copy(gacc, scr)
                else:
                    nc.vector.scalar_tensor_tensor(
                        out=gacc,
                        in0=scr,
                        scalar=1.0,
                        in1=gacc,
                        op0=mybir.AluOpType.mult,
                        op1=mybir.AluOpType.add,
                    )

            g_sb = [None] * MT

            def emit_gate():
                for m in range(MT):
                    # softmax with bias folded in multiplicatively:
                    # g = exp(l)*exp(b) / sum_e exp(l)*exp(b)
                    p_t = gpool.tile([P, E], F32, tag=f"p{m}", name=f"p{m}")
                    # exp(logits); |logits| <~ 3 so no max-subtraction needed
                    nc.scalar.activation(
                        p_t, gacc[:, m * E:(m + 1) * E],
                        mybir.ActivationFunctionType.Exp,
                    )
                    q_t = gpool.tile([P, E], F32, tag=f"q{m}", name=f"q{m}")
                    s_t = gpool.tile([P, 1], F32, tag=f"s{m}", name=f"s{m}")
                    # q = p * expb (and s = sum_e q in the same op)
                    nc.vector.scalar_tensor_tensor(
                        out=q_t,
                        in0=p_t,
                        scalar=1.0,
                        in1=expb_sb,
                        op0=mybir.AluOpType.mult,
                        op1=mybir.AluOpType.mult,
                        accum_out=s_t,
                    )
                    rs_t = gpool.tile([P, 1], F32, tag=f"rs{m}", name=f"rs{m}")
                    nc.vector.reciprocal(rs_t, s_t)
                    g_t = gpool.tile([P, E], F32, tag=f"g{m}", name=f"g{m}")
                    nc.vector.tensor_scalar_mul(g_t, q_t, rs_t)
                    g_sb[m] = g_t

            # ---- main: all-expert GEMM + fused gate combine ---------------
            for n in range(NT):
                acc = [None] * MT
                for e in range(E):
                    # one 1MiB DMA per (n, e): all k-tiles of this d_out
                    # slice. (n==0, e==0) was loaded k-granular at the head.
                    is_final = (n == NT - 1 and e == E - 1)
                    if n == 0 and e == 0:
                        wt = wt0
                    else:
                        wt = wpool.tile([P, KT, NO], BF16, tag="w")
                        if n == 0 and e == 1:
                            # expert 1 is head-critical: chain it on
                            # x_k45's arrival (so it doesn't steal ladder
                            # bandwidth) and use HWDGE (SWDGE would land
                            # it ~10us/MB later). All later experts ride
                            # the SWDGE drip: its gentle ~90GB/s delivery
                            # doesn't contend with the PE's SBUF reads,
                            # while HWDGE's ~300GB/s bursts stretch every
                            # in-flight matmul (measured +48ns/MM).
                            nc.gpsimd.tensor_copy(
                                wt[0:1, 2:3, 0:1], xtiles[4][0:1, 0:1, 0:1]
                            )
                            eng = nc.scalar
                        else:
                            eng = nc.gpsimd
                        if is_final:
                            # final block stays full-bf16 (accuracy head-
                            # room is spent on the 14 middle blocks)
                            eng.dma_start(
                                out=wt,
                                in_=w_d[e, :, n * NO:(n + 1) * NO].rearrange(
                                    "(k p) o -> p k o", p=P
                                ),
                            )
                        else:
                            # middle block: k-tiles 0-1 ride the fp8
                            # DoubleRow pass, so only k2-7 in bf16
                            wt8 = w8pool.tile([P, 2, NO], FP8, tag="w8")
                            if n == 0 and e == 1:
                                nc.gpsimd.tensor_copy(
                                    wt8[0:1, 0:1, 0:1],
                                    xtiles[4][0:1, 0:1, 0:1],
                                )
                            eng.dma_start(
                                out=wt8,
                                in_=w8_d[e, :, :, n * NO:(n + 1) * NO],
                            )
                            eng.dma_start(
                                out=wt[:, 2:, :],
                                in_=w_d[
                                    e, 2 * P:, n * NO:(n + 1) * NO
                                ].rearrange("(k p) o -> p k o", p=P),
                            )
                    # Expert 0 (head, DMA-paced): k-outer so the PE can
                    # consume each arriving k chunk across all m groups.
                    # Experts 1+: m-outer — each PSUM group is 8 consecutive
                    # matmuls, slots cycle fast, and the per-m combine +
                    # output DMA spread across the stream.
                    if n == 0 and e == 0:
                        psy_l = [None] * MT
                        for k in range(KT):
                            # gate section first: 8 tiny matmuls whose
                            # inputs (gw + chunk k) are already resident —
                            # extra ready work that absorbs DMA jitter
                            emit_gate_k(k)
                            for m in range(MT - 1):
                                if k == 0:
                                    psy_l[m] = ps.tile(
                                        [P, NO], F32, tag="ps", name=f"psk{m}"
                                    )
                                nc.tensor.matmul(
                                    psy_l[m],
                                    lhsT=xT(k)[:, m * P:(m + 1) * P],
                                    rhs=wt[:, k, :],
                                    start=(k == 0),
                                    stop=(k == KT - 1),
                                )
                        psy_l[MT - 1] = ps.tile(
                            [P, NO], F32, tag="ps", name="psk_last"
                        )
                        for k in range(KT):
                            nc.tensor.matmul(
                                psy_l[MT - 1],
                                lhsT=xT(k)[:, (MT - 1) * P:MT * P],
                                rhs=wt[:, k, :],
                                start=(k == 0),
                                stop=(k == KT - 1),
                            )
                        # copies first (no gate dep — frees the PSUM slots
                        # even though the gate hasn't run), then the gate,
                        # then the g0 scale-muls which need it
                        for m in range(MT):
                            acc[m] = accp.tile(
                                [P, NO], F32, tag="acc", name=f"acc{m}"
                            )
                            nc.vector.tensor_copy(acc[m], psy_l[m])
                        emit_gate()
                        for m in range(MT):
                            nc.vector.tensor_scalar_mul(
                                acc[m], acc[m], g_sb[m][:, 0:1]
                            )
                        continue
                    if n == NT - 1 and e == E - 1:
                        # final block: every m-group as two N=256 PSUM
                        # half-groups so each half's combine + 128KB output
                        # DMA launches as soon as that half completes —
                        # the output stream drains alongside the matmuls
                        # instead of piling up after the last one
                        for m in range(MT):
                            # the very last m-group goes quarter-granular
                            # (N=128): the final combine shrinks 483->~300ns
                            # and the final DMA transfer halves — both sit
                            # on the kernel's critical tail
                            nsplit = 4 if m == MT - 1 else 2
                            NH = NO // nsplit
                            for h in range(nsplit):
                                psy_h = ps.tile(
                                    [P, NH], F32, tag="ps",
                                    name=f"psyh{m}_{h}"
                                )
                                for k in range(KT):
                                    nc.tensor.matmul(
                                        psy_h,
                                        lhsT=xT(k)[:, m * P:(m + 1) * P],
                                        rhs=wt[:, k, h * NH:(h + 1) * NH],
                                        start=(k == 0),
                                        stop=(k == KT - 1),
                                    )
                                nc.vector.scalar_tensor_tensor(
                                    out=acc[m][:, h * NH:(h + 1) * NH],
                                    in0=psy_h,
                                    scalar=g_sb[m][:, e:e + 1],
                                    in1=acc[m][:, h * NH:(h + 1) * NH],
                                    op0=mybir.AluOpType.mult,
                                    op1=mybir.AluOpType.add,
                                )
                                eng = nc.scalar if h % 2 == 0 else nc.sync
                                off = m * P * O + (n * NO + h * NH) * P
                                eng.dma_start(
                                    out=out_d[off:off + P * NH].rearrange(
                                        "(p c) -> p c", p=P
                                    ),
                                    in_=acc[m][:, h * NH:(h + 1) * NH],
                                )
                        continue
                    # k-tiles 0-1 ride one fp8 DoubleRow matmul per m
                    # (2 weights/cell -> K=256 per pass); k2-7 accumulate
                    # on top in bf16. NOTE: grouping consecutive DR
                    # matmuls (to pull LDWEIGHTS ahead) wedges the PE
                    # (NRT_EXEC_UNIT_UNRECOVERABLE) — keep them
                    # interleaved one per m-group.
                    for m in range(MT):
                        psy = ps.tile([P, NO], F32, tag="ps", name=f"psy{m}")
                        nc.tensor.matmul(
                            psy,
                            lhsT=x8_sb[:, :, m * P:(m + 1) * P],
                            rhs=wt8[:, :, :],
                            start=True,
                            stop=False,
                            perf_mode=mybir.MatmulPerfMode.DoubleRow,
                        )
                        for k in range(2, KT):
                            nc.tensor.matmul(
                                psy,
                                lhsT=xT(k)[:, m * P:(m + 1) * P],
                                rhs=wt[:, k, :],
                                start=False,
                                stop=(k == KT - 1),
                            )
                        if e == 0:
                            # init acc with an unscaled copy (no gate dep —
                            # frees the PSUM slot even if the gate is still
                            # running), then fold g0 in as a separate op
                            acc[m] = accp.tile(
                                [P, NO], F32, tag="acc", name=f"acc{m}"
                            )
                            nc.vector.tensor_copy(acc[m], psy)
                            nc.vector.tensor_scalar_mul(
                                acc[m], acc[m], g_sb[m][:, 0:1]
                            )
                        else:
                            nc.vector.scalar_tensor_tensor(
                                out=acc[m],
                                in0=psy,
                                scalar=g_sb[m][:, e:e + 1],
                                in1=acc[m],
                                op0=mybir.AluOpType.mult,
                                op1=mybir.AluOpType.add,
                            )
                        if e == E - 1:
                            eng = nc.sync if m % 2 == 0 else nc.scalar
                            off = m * P * O + n * NO * P
                            eng.dma_start(
                                out=out_d[off:off + P * NO].rearrange(
                                    "(p c) -> p c", p=P
                                ),
                                in_=acc[m],
                            )
    nc.compile()
    return nc


# ---------------------------------------------------------------------------
# Host wrapper: full inputs -> shard -> run SPMD on 8 cores -> gather
# ---------------------------------------------------------------------------

N_CORES = 8
_B, _S, _DIN, _DOUT, _E = 4, 2048, 1024, 1024, 8


def _host_gwt(gate_w):
    """[D, E] -> [128, KT*E] with gwt[p, k*E+e] = gate_w[k*128+p, e]."""
    D, E = gate_w.shape
    kt = D // P
    return np.ascontiguousarray(
        gate_w.reshape(kt, P, E).transpose(1, 0, 2).reshape(P, kt * E)
    )


LAST_RESULTS = None  # BassKernelResults of the most recent run (for profiling)


def kernel(x, gate_w, gate_b, expert_w, _trace=False):
    global LAST_RESULTS
    from concourse.bass_utils import run_bass_kernel_spmd

    x = np.asarray(x)
    tokens = x.reshape(-1, _DIN)  # [8192, 1024]
    n_tok = tokens.shape[0]
    tpc = n_tok // N_CORES  # tokens per core

    # ALL weights are shipped scaled by 64 (exact power-of-2, so the bf16
    # path is bit-identical after the host-side /64 of the output) to put
    # the fp8 copies of k-tiles 0-1 in e4m3's normal range.
    w64 = np.asarray(expert_w, dtype=np.float32) * np.float32(64.0)
    w_bf = w64.astype(ml_dtypes.bfloat16)
    # w8[e, p, j, o] = e4m3(64*w[e, j*128+p, o]) for k-tiles j=0,1
    w8 = np.ascontiguousarray(
        w64[:, : 2 * P, :]
        .reshape(_E, 2, P, _DOUT)
        .transpose(0, 2, 1, 3)
    ).astype(ml_dtypes.float8_e4m3)
    gwt_bf = _host_gwt(np.asarray(gate_w)).astype(ml_dtypes.bfloat16)
    expb = np.broadcast_to(
        np.exp(np.asarray(gate_b, dtype=np.float64)).astype(np.float32),
        (P, _E),
    ).copy()

    in_maps = []
    for c in range(N_CORES):
        shard = tokens[c * tpc:(c + 1) * tpc]  # [1024, 1024]
        xT32 = np.ascontiguousarray(shard.T)   # [1024(D), 1024(T)] f32
        xT = xT32.astype(ml_dtypes.bfloat16)
        # x8T[p, j, t] = e4m3(x[t, j*128+p]) for k-tiles j=0,1
        x8T = np.ascontiguousarray(
            xT32[: 2 * P].reshape(2, P, tpc).transpose(1, 0, 2)
        ).astype(ml_dtypes.float8_e4m3)
        in_maps.append(
            {"xT": xT, "x8T": x8T, "w": w_bf, "w8": w8,
             "gwt": gwt_bf, "expb": expb}
        )

    nc = build_moe_nc(T=tpc, D=_DIN, O=_DOUT, E=_E)
    res = run_bass_kernel_spmd(nc, in_maps, list(range(N_CORES)), trace=_trace)
    LAST_RESULTS = res
    # reassemble the flat tile-major output (must mirror the kernel's
    # tile map: n0 full-width per m; final block halves, m7 quarters)
    tile_map = [(m, 0, 512) for m in range(8)]
    for m in range(7):
        tile_map += [(m, 512, 256), (m, 768, 256)]
    tile_map += [(7, 512 + q * 128, 128) for q in range(4)]
    tpc_o = tpc * _DOUT
    parts = []
    for c in range(N_CORES):
        flat = np.asarray(res.results[c]["out"]).reshape(tpc_o)
        full_c = np.empty((tpc, _DOUT), dtype=np.float32)
        for (m, c0, ncol) in tile_map:
            off = m * P * _DOUT + c0 * P
            full_c[m * P:(m + 1) * P, c0:c0 + ncol] = flat[
                off:off + P * ncol
            ].reshape(P, ncol)
        parts.append(full_c)
    full = np.concatenate(parts, axis=0).astype(np.float32)
    full *= np.float32(1.0 / 64.0)  # undo the x64 weight scaling (exact)
    return full.reshape(_B, _S, _DOUT)



# revision 27
# speedup vs baseline: 1.0150x; 1.0089x over previous
"""MoE layer (dense all-expert routing) Trainium2 Bass kernel.

Problem: x[4,2048,1024] f32, gate_w[1024,8], gate_b[8], expert_w[8,1024,1024].
  gate = softmax(x @ gate_w + gate_b)                  # [B,S,E]
  out  = einsum('bse,bseo->bso', gate, einsum('bsi,eio->bseo', x, expert_w))

Sharding: data-parallel over tokens. 8192 tokens split into 8 shards of 1024;
each core computes its shard against all 8 experts (weights replicated).
No collectives; host concatenates shard outputs.

Per-core kernel (tuned against perfetto traces; 250.5us -> 242us bf16
baseline -> ~220us mixed-precision):
  - mixed precision: k-tiles 0-1 of the contraction run as ONE fp8-e4m3
    DoubleRow matmul per (n,e,m) group (the PE packs 2 weights/cell ->
    K=256 per pass) for the 14 middle (n,e) blocks; k2-7 and the head/
    final blocks stay bf16; gate fully bf16. Host-measured = HW-measured
    rel err 1.774e-2 (budget 2e-2; error scales as 0.0376*sqrt(alpha),
    alpha = fp8 fraction = 14/16 * 1/4). ALL weights shipped x64
    (exact power-of-2; puts e4m3 w in normal range) and the output is
    divided by 64 on the host.
  - the DR matmul's 256-col LDWEIGHTS is only partially hidden (slot
    ~408ns vs 2x215.4ns for the bf16 pair it replaces -> ~186ns saved
    per m-group). GROUPING consecutive DR matmuls to hide the loads
    wedges the PE (NRT_EXEC_UNIT_UNRECOVERABLE) — keep 1 DR per group.
  - all other matmuls bf16 with f32 PSUM accumulation
  - head: all in-flight DMAs round-robin and complete together near the
    end of the aggregate transfer (+~2.3us trigger/completion latency
    each), so the input load is an arrival-paced LADDER: a minimal first
    wave (gate weights, x_k0 split across both HWDGE rings, w0's k0
    slice), then one-step-lookahead waves chained on earlier chunks'
    completions. Chains are pure Tile deps: a 1-element gpsimd copy from
    the trigger chunk into the next chunk's tile makes the next
    dma_start a WAW-ordered later writer.
  - ~50 N=128 warm-up matmuls bridge sequencer-start -> first-data and
    hold the HAM clock gate at 8/8 (a >1.7us idle gap re-throttles to
    1.2GHz for ~3.4us; x_k0 arrival jitters 12.4-14.3us so the bridge
    overshoots)
  - gate: logits accumulated per k-section (8 single-matmul PSUM groups
    into a scratch bank — start+stop on every matmul, immune to the
    whole-bank has_written clear — then one DVE op folds the scratch
    into an SBUF accumulator), interleaved with expert 0's k-sections so
    each section needs only the k-th x chunk. Bias applied as a
    host-precomputed exp(b) columnwise multiply after exp; softmax =
    exp, fused mul+sum, reciprocal, scale.
  - main loop: per (n-half, expert) one 1MiB weight DMA on the gpsimd
    SWDGE ring (its gentle ~90GB/s drip doesn't contend with the PE's
    SBUF reads; only head-critical W1 rides HWDGE, chained on x_k45),
    64 matmuls, then per m one fused DVE op: acc = (psum*g[:,e]) + acc
  - final (n,e) block runs every m-group as two N=256 PSUM half-groups
    (the very last m-group as four N=128 quarters) so each output DMA
    launches well before the last matmul and the critical tail shrinks
  - steady state measured at the stream floor: 215-216ns per N=512
    matmul, zero PE gaps; remaining costs are the ~8us sequencer boot,
    ~4.4us head DMA latency, ~4.3us tail (output DMA completion +
    epilogue barrier), and a ~160ns stall every 10.79us (HBM refresh-
    like artifact)
"""

import numpy as np
import ml_dtypes
from contextlib import ExitStack

import concourse.bacc as bacc
import concourse.bass as bass
import concourse.mybir as mybir
import concourse.tile as tile

BF16 = mybir.dt.bfloat16
F32 = mybir.dt.float32
FP8 = mybir.dt.float8e4  # TRN FP8_EXP4 == ml_dtypes.float8_e4m3 (IEEE, max +-240)

P = 128  # partitions


def build_moe_nc(T=1024, D=1024, O=1024, E=8, NO=512, w_bufs=2, acc_bufs=16):
    """Build the per-core Bass program.

    T: tokens per core, D: d_in, O: d_out, E: experts, NO: d_out tile (<=512).
    """
    KT = D // P   # k tiles (contraction)
    MT = T // P   # token tiles
    NT = O // NO  # d_out tiles

    nc = bacc.Bacc("TRN2", target_bir_lowering=False, debug=False)
    xT_d = nc.dram_tensor("xT", [D, T], BF16, kind="ExternalInput")
    w_d = nc.dram_tensor("w", [E, D, O], BF16, kind="ExternalInput")
    # fp8 copies of the first two k-tiles for the DoubleRow (2x) passes.
    # x8T holds the stationary operand in DoubleRowSwInterleave layout
    # (per m-group: A/B k-pairs interleaved per column, columns reversed —
    # contiguous weight reads, so LDWEIGHTS can use the fast path):
    #   x8T[p, g, 2*(127-u)+j] = e4m3(x[g*128+u, j*128+p])
    # w8 (moving operand) keeps the plain [k-pair, o] layout:
    #   w8[e, p, j, o] = e4m3(64*w[e, j*128+p, o])
    x8T_d = nc.dram_tensor("x8T", [P, T // P, 2 * P], FP8, kind="ExternalInput")
    w8_d = nc.dram_tensor("w8", [E, P, 2, O], FP8, kind="ExternalInput")
    # gwt[p, k*E+e] = gate_w[k*128+p, e]  (host pre-tiled, contiguous DMA)
    gwt_d = nc.dram_tensor("gwt", [P, KT * E], BF16, kind="ExternalInput")
    # expb[p, e] = exp(gate_b[e]) replicated across partitions
    expb_d = nc.dram_tensor("expb", [P, E], F32, kind="ExternalInput")
    # flat tile-major output: each DMA writes one CONTIGUOUS HBM block
    # (vs 128 rows strided 4KB apart) — fewer descriptors, cheaper
    # trigger gen + WAW completion on the kernel's critical tail.
    # span for tile (m, col0, ncols): [m*P*O + col0*P, + P*ncols)
    out_d = nc.dram_tensor("out", [T * O], F32, kind="ExternalOutput")

    with tile.TileContext(nc) as tc:
        with ExitStack() as ctx:
            singles = ctx.enter_context(tc.tile_pool(name="singles", bufs=1))
            wpool = ctx.enter_context(tc.tile_pool(name="w", bufs=w_bufs))
            w8pool = ctx.enter_context(tc.tile_pool(name="w8", bufs=w_bufs))
            accp = ctx.enter_context(tc.tile_pool(name="acc", bufs=acc_bufs))
            gpool = ctx.enter_context(tc.tile_pool(name="gate", bufs=1))
            ps = ctx.enter_context(tc.tile_pool(name="ps", bufs=7, space="PSUM"))

            # ---- resident loads -------------------------------------------
            # HAM warm-up: short N=128 dummy matmuls on a memset tile keep
            # the PE busy from sequencer-start until the first x chunk
            # lands (~4us: DMA trigger + ~2.3us completion latency +
            # transfer), so the clock gate is at 8/8 when real work starts
            # and the first real matmuls run at full clock.
            warm = singles.tile([P, P], BF16, tag="warm")
            nc.vector.memset(warm, 0.0)
            psw = ps.tile([P, P], F32, tag="psg", bufs=1, name="psw")
            # NWARM=26 measured best (219.7us): the stream starts ~11.2us
            # slightly cold. Both a longer bridge (40 -> stream warm from
            # 11.7 but the ladder starves k1-k3, 220.9us) and more
            # aggressive ladders (mid-stream HAM re-throttle, 221.9-222.2)
            # measured worse — the ladder can't feed a warm PE earlier.
            NWARM = 26
            for j in range(NWARM):
                nc.tensor.matmul(
                    psw, lhsT=warm, rhs=warm,
                    start=(j == 0), stop=(j == NWARM - 1),
                )

            # Head loads, all on the two HWDGE rings, staged as an
            # arrival-paced LADDER. All concurrently in-flight DMAs
            # round-robin at packet granularity and complete together near
            # the end of the aggregate transfer (measured: every head DMA
            # of an all-at-once plan lands at 20-27us), so instead each
            # wave's trigger is chained on the previous x chunk's
            # completion. The chain is expressed as a pure Tile dep: a
            # 1-element DVE copy from the previous chunk into the next
            # chunk's tile forces the next dma_start (a WAW-ordered later
            # writer) to wait for the copy, which waits for the data.
            wt0 = wpool.tile([P, KT, NO], BF16, tag="w", name="wt0")
            w0_src = w_d[0, :, 0:NO].rearrange("(k p) o -> p k o", p=P)
            xparts = {}
            xtiles = {}

            def chain_src(chain_on):
                # chain_on: chunk index, or (chunk index, token col)
                kc, col = chain_on if isinstance(chain_on, tuple) else (
                    chain_on, 0
                )
                return xtiles[kc][0:1, 0:1, col:col + 1]

            def load_x(kc, nk, eng, chain_on=None):
                t = singles.tile(
                    [P, nk, T], BF16, tag=f"xT{kc}", name=f"xc{kc}"
                )
                if chain_on is not None:
                    # chain copies ride the otherwise-idle gpsimd engine —
                    # on the in-order DVE they'd block the gate's
                    # accumulate ops behind later rungs of the ladder
                    nc.gpsimd.tensor_copy(
                        t[0:1, 0:1, 0:1], chain_src(chain_on)
                    )
                eng.dma_start(
                    out=t,
                    in_=xT_d[kc * P:(kc + nk) * P, :].rearrange(
                        "(k p) t -> p k t", p=P
                    ),
                )
                xtiles[kc] = t
                for i in range(nk):
                    xparts[kc + i] = (t, i)

            def load_w0(kc, nk, eng, chain_on=None):
                if chain_on is not None:
                    nc.gpsimd.tensor_copy(
                        wt0[0:1, kc:kc + 1, 0:1], chain_src(chain_on)
                    )
                eng.dma_start(
                    out=wt0[:, kc:kc + nk, :], in_=w0_src[:, kc:kc + nk, :]
                )

            # wave 1 (in flight immediately, ~420KB): consts + x_k0 split
            # across BOTH rings + w0's k0 slice — the minimal set for the
            # gate-k0 + e0-k0 sections, so the PE's first real work starts
            # as early as the DMA path allows. Later waves are chained
            # with one-step LOOKAHEAD (rung i+1 fires on rung i-1's
            # completion) so the ~2-3us per-DMA trigger+completion latency
            # pipelines away while keeping at most ~1.5MB in flight (full
            # concurrency degrades to everything-lands-at-the-end
            # round-robin).
            # x_k0's halves trigger FIRST on each ring — each DIRECT2D
            # descriptor-gen costs ~0.65us of sequencer time, so consts
            # queued ahead of x would delay the whole pipeline start
            # the first wave rides THREE parallel queues: x_k0 half A on
            # the sync ring, gate weights + x_k0 half B on the scalar
            # ring, w0_k0/k1 on the SWDGE ring (its ~1us first-byte is
            # fine — they're consumed after the gate-k0 section). Rungs
            # are emitted in FIRE-TIME order per queue (each queue is
            # strict FIFO; a rung queued behind a later-firing one waits)
            TH = T // 2
            xc0 = singles.tile([P, 1, T], BF16, tag="xT0", name="xc0")
            nc.sync.dma_start(out=xc0[:, 0, 0:TH], in_=xT_d[0:P, 0:TH])
            nc.scalar.dma_start(out=xc0[:, 0, TH:T], in_=xT_d[0:P, TH:T])
            xtiles[0] = xc0
            xparts[0] = (xc0, 0)
            gw_t = singles.tile([P, KT, E], BF16, tag="gw")
            nc.gpsimd.dma_start(
                out=gw_t, in_=gwt_d[:, :].rearrange("p (k e) -> p k e", e=E)
            )
            expb_sb = singles.tile([P, E], F32, tag="expb")
            nc.scalar.dma_start(out=expb_sb, in_=expb_d[:, :])
            load_w0(0, 1, nc.gpsimd)
            # rung chains: key (tile, token col) identifies the half/chunk
            # whose completion triggers the rung
            load_x(1, 1, nc.scalar, chain_on=0)          # on x_k0 half A
            load_w0(1, 1, nc.sync, chain_on=0)
            load_x(2, 2, nc.sync, chain_on=(0, TH))      # on x_k0 half B
            load_w0(2, 2, nc.scalar, chain_on=(0, TH))
            load_x(4, 2, nc.sync, chain_on=(0, TH))      # on x_k0 half B
            load_w0(4, 2, nc.scalar, chain_on=1)         # on x_k1
            load_x(6, 2, nc.sync, chain_on=1)            # on x_k1
            # w0_k67 isn't consumed until ~11us after the first section —
            # chain it deepest so its 256KB doesn't dilute x's bandwidth
            # share in the early delivery window
            load_w0(6, 2, nc.scalar, chain_on=4)         # on x_k45
            # fp8 copy of x k-tiles 0-1 for the DoubleRow passes; first
            # consumed by block (n0, e1) ~16us after the stream starts, so
            # chain it deep — and BEFORE the e1 weight chain copies, which
            # fire on the same arrival (gpsimd FIFO order)
            x8_sb = singles.tile([P, MT, 2 * P], FP8, tag="x8", name="x8")
            nc.gpsimd.tensor_copy(x8_sb[0:1, 0:1, 0:1], chain_src(4))
            nc.sync.dma_start(out=x8_sb, in_=x8T_d[:, :, :])

            def xT(k):
                t, i = xparts[k]
                return t[:, i, :]

            # ---- gate ------------------------------------------------------
            # Logits are accumulated per k-section so each section only
            # needs the k-th x chunk: 8 single-matmul PSUM groups (start+
            # stop on every matmul — immune to the whole-bank has_written
            # clear that start=True performs, so no cross-group corruption)
            # into a scratch bank, then one DVE op folds the scratch into
            # an SBUF accumulator. Interleaved with expert 0's k-sections.
            gacc = gpool.tile([P, MT * E], F32, tag="gacc", name="gacc")

            def emit_gate_k(k):
                scr = ps.tile(
                    [P, MT * E], F32, tag="psg", bufs=1, name=f"gsc{k}"
                )
                for m in range(MT):
                    nc.tensor.matmul(
                        scr[:, m * E:(m + 1) * E],
                        lhsT=xT(k)[:, m * P:(m + 1) * P],
                        rhs=gw_t[:, k, :],
                        start=True,
                        stop=True,
                    )
                if k == 0:
                    nc.vector.tensor_copy(gacc, scr)
                else:
                    nc.vector.scalar_tensor_tensor(
                        out=gacc,
                        in0=scr,
                        scalar=1.0,
                        in1=gacc,
                        op0=mybir.AluOpType.mult,
                        op1=mybir.AluOpType.add,
                    )

            g_sb = [None] * MT

            def emit_gate():
                for m in range(MT):
                    # softmax with bias folded in multiplicatively:
                    # g = exp(l)*exp(b) / sum_e exp(l)*exp(b)
                    p_t = gpool.tile([P, E], F32, tag=f"p{m}", name=f"p{m}")
                    # exp(logits); |logits| <~ 3 so no max-subtraction needed
                    nc.scalar.activation(
                        p_t, gacc[:, m * E:(m + 1) * E],
                        mybir.ActivationFunctionType.Exp,
                    )
                    q_t = gpool.tile([P, E], F32, tag=f"q{m}", name=f"q{m}")
                    s_t = gpool.tile([P, 1], F32, tag=f"s{m}", name=f"s{m}")
                    # q = p * expb (and s = sum_e q in the same op)
                    nc.vector.scalar_tensor_tensor(
                        out=q_t,
                        in0=p_t,
                        scalar=1.0,
                        in1=expb_sb,
                        op0=mybir.AluOpType.mult,
                        op1=mybir.AluOpType.mult,
                        accum_out=s_t,
                    )
                    rs_t = gpool.tile([P, 1], F32, tag=f"rs{m}", name=f"rs{m}")
                    nc.vector.reciprocal(rs_t, s_t)
                    g_t = gpool.tile([P, E], F32, tag=f"g{m}", name=f"g{m}")
                    nc.vector.tensor_scalar_mul(g_t, q_t, rs_t)
                    g_sb[m] = g_t

            # ---- main: all-expert GEMM + fused gate combine ---------------
            for n in range(NT):
                acc = [None] * MT
                for e in range(E):
                    # one 1MiB DMA per (n, e): all k-tiles of this d_out
                    # slice. (n==0, e==0) was loaded k-granular at the head.
                    is_final = (n == NT - 1 and e == E - 1)
                    if n == 0 and e == 0:
                        wt = wt0
                    else:
                        wt = wpool.tile([P, KT, NO], BF16, tag="w")
                        if n == 0 and e == 1:
                            # expert 1 is head-critical: chain it on
                            # x_k45's arrival (so it doesn't steal ladder
                            # bandwidth) and use HWDGE (SWDGE would land
                            # it ~10us/MB later). All later experts ride
                            # the SWDGE drip: its gentle ~90GB/s delivery
                            # doesn't contend with the PE's SBUF reads,
                            # while HWDGE's ~300GB/s bursts stretch every
                            # in-flight matmul (measured +48ns/MM).
                            nc.gpsimd.tensor_copy(
                                wt[0:1, 2:3, 0:1], xtiles[4][0:1, 0:1, 0:1]
                            )
                            eng = nc.scalar
                        else:
                            eng = nc.gpsimd
                        if is_final:
                            # final block stays full-bf16 (accuracy head-
                            # room is spent on the 14 middle blocks)
                            eng.dma_start(
                                out=wt,
                                in_=w_d[e, :, n * NO:(n + 1) * NO].rearrange(
                                    "(k p) o -> p k o", p=P
                                ),
                            )
                        else:
                            # middle block: k-tiles 0-1 ride the fp8
                            # DoubleRow pass, so only k2-7 in bf16
                            wt8 = w8pool.tile([P, 2, NO], FP8, tag="w8")
                            if n == 0 and e == 1:
                                nc.gpsimd.tensor_copy(
                                    wt8[0:1, 0:1, 0:1],
                                    xtiles[4][0:1, 0:1, 0:1],
                                )
                            eng.dma_start(
                                out=wt8,
                                in_=w8_d[e, :, :, n * NO:(n + 1) * NO],
                            )
                            eng.dma_start(
                                out=wt[:, 2:, :],
                                in_=w_d[
                                    e, 2 * P:, n * NO:(n + 1) * NO
                                ].rearrange("(k p) o -> p k o", p=P),
                            )
                    # Expert 0 (head, DMA-paced): k-outer so the PE can
                    # consume each arriving k chunk across all m groups.
                    # Experts 1+: m-outer — each PSUM group is 8 consecutive
                    # matmuls, slots cycle fast, and the per-m combine +
                    # output DMA spread across the stream.
                    if n == 0 and e == 0:
                        psy_l = [None] * MT
                        for k in range(KT):
                            # gate section first: 8 tiny matmuls whose
                            # inputs (gw + chunk k) are already resident —
                            # extra ready work that absorbs DMA jitter
                            emit_gate_k(k)
                            for m in range(MT - 1):
                                if k == 0:
                                    psy_l[m] = ps.tile(
                                        [P, NO], F32, tag="ps", name=f"psk{m}"
                                    )
                                nc.tensor.matmul(
                                    psy_l[m],
                                    lhsT=xT(k)[:, m * P:(m + 1) * P],
                                    rhs=wt[:, k, :],
                                    start=(k == 0),
                                    stop=(k == KT - 1),
                                )
                        psy_l[MT - 1] = ps.tile(
                            [P, NO], F32, tag="ps", name="psk_last"
                        )
                        for k in range(KT):
                            nc.tensor.matmul(
                                psy_l[MT - 1],
                                lhsT=xT(k)[:, (MT - 1) * P:MT * P],
                                rhs=wt[:, k, :],
                                start=(k == 0),
                                stop=(k == KT - 1),
                            )
                        # copies first (no gate dep — frees the PSUM slots
                        # even though the gate hasn't run), then the gate,
                        # then the g0 scale-muls which need it
                        for m in range(MT):
                            acc[m] = accp.tile(
                                [P, NO], F32, tag="acc", name=f"acc{m}"
                            )
                            nc.vector.tensor_copy(acc[m], psy_l[m])
                        emit_gate()
                        for m in range(MT):
                            nc.vector.tensor_scalar_mul(
                                acc[m], acc[m], g_sb[m][:, 0:1]
                            )
                        continue
                    if n == NT - 1 and e == E - 1:
                        # final block: every m-group as two N=256 PSUM
                        # half-groups so each half's combine + 128KB output
                        # DMA launches as soon as that half completes —
                        # the output stream drains alongside the matmuls
                        # instead of piling up after the last one
                        for m in range(MT):
                            # the very last m-group goes quarter-granular
                            # (N=128): the final combine shrinks 483->~300ns
                            # and the final DMA transfer halves — both sit
                            # on the kernel's critical tail
                            nsplit = 4 if m == MT - 1 else 2
                            NH = NO // nsplit
                            for h in range(nsplit):
                                psy_h = ps.tile(
                                    [P, NH], F32, tag="ps",
                                    name=f"psyh{m}_{h}"
                                )
                                for k in range(KT):
                                    nc.tensor.matmul(
                                        psy_h,
                                        lhsT=xT(k)[:, m * P:(m + 1) * P],
                                        rhs=wt[:, k, h * NH:(h + 1) * NH],
                                        start=(k == 0),
                                        stop=(k == KT - 1),
                                    )
                                nc.vector.scalar_tensor_tensor(
                                    out=acc[m][:, h * NH:(h + 1) * NH],
                                    in0=psy_h,
                                    scalar=g_sb[m][:, e:e + 1],
                                    in1=acc[m][:, h * NH:(h + 1) * NH],
                                    op0=mybir.AluOpType.mult,
                                    op1=mybir.AluOpType.add,
                                )
                                eng = nc.scalar if h % 2 == 0 else nc.sync
                                off = m * P * O + (n * NO + h * NH) * P
                                eng.dma_start(
                                    out=out_d[off:off + P * NH].rearrange(
                                        "(p c) -> p c", p=P
                                    ),
                                    in_=acc[m][:, h * NH:(h + 1) * NH],
                                )
                        continue
                    # k-tiles 0-1 ride one fp8 DoubleRow matmul per m
                    # (2 weights/cell -> K=256 per pass); k2-7 accumulate
                    # on top in bf16. NOTE: grouping consecutive DR
                    # matmuls (to pull LDWEIGHTS ahead) wedges the PE
                    # (NRT_EXEC_UNIT_UNRECOVERABLE) — keep them
                    # interleaved one per m-group.
                    for m in range(MT):
                        psy = ps.tile([P, NO], F32, tag="ps", name=f"psy{m}")
                        nc.tensor.matmul(
                            psy,
                            lhsT=x8_sb[:, m, :],
                            rhs=wt8[:, :, :],
                            start=True,
                            stop=False,
                            perf_mode=mybir.MatmulPerfMode.DoubleRowSwInterleave,
                        )
                        for k in range(2, KT):
                            nc.tensor.matmul(
                                psy,
                                lhsT=xT(k)[:, m * P:(m + 1) * P],
                                rhs=wt[:, k, :],
                                start=False,
                                stop=(k == KT - 1),
                            )
                        if e == 0:
                            # init acc with an unscaled copy (no gate dep —
                            # frees the PSUM slot even if the gate is still
                            # running), then fold g0 in as a separate op
                            acc[m] = accp.tile(
                                [P, NO], F32, tag="acc", name=f"acc{m}"
                            )
                            nc.vector.tensor_copy(acc[m], psy)
                            nc.vector.tensor_scalar_mul(
                                acc[m], acc[m], g_sb[m][:, 0:1]
                            )
                        else:
                            nc.vector.scalar_tensor_tensor(
                                out=acc[m],
                                in0=psy,
                                scalar=g_sb[m][:, e:e + 1],
                                in1=acc[m],
                                op0=mybir.AluOpType.mult,
                                op1=mybir.AluOpType.add,
                            )
                        if e == E - 1:
                            eng = nc.sync if m % 2 == 0 else nc.scalar
                            off = m * P * O + n * NO * P
                            eng.dma_start(
                                out=out_d[off:off + P * NO].rearrange(
                                    "(p c) -> p c", p=P
                                ),
                                in_=acc[m],
                            )
    nc.compile()
    return nc


# ---------------------------------------------------------------------------
# Host wrapper: full inputs -> shard -> run SPMD on 8 cores -> gather
# ---------------------------------------------------------------------------

N_CORES = 8
_B, _S, _DIN, _DOUT, _E = 4, 2048, 1024, 1024, 8


def _host_gwt(gate_w):
    """[D, E] -> [128, KT*E] with gwt[p, k*E+e] = gate_w[k*128+p, e]."""
    D, E = gate_w.shape
    kt = D // P
    return np.ascontiguousarray(
        gate_w.reshape(kt, P, E).transpose(1, 0, 2).reshape(P, kt * E)
    )


LAST_RESULTS = None  # BassKernelResults of the most recent run (for profiling)


def kernel(x, gate_w, gate_b, expert_w, _trace=False):
    global LAST_RESULTS
    from concourse.bass_utils import run_bass_kernel_spmd

    x = np.asarray(x)
    tokens = x.reshape(-1, _DIN)  # [8192, 1024]
    n_tok = tokens.shape[0]
    tpc = n_tok // N_CORES  # tokens per core

    # ALL weights are shipped scaled by 64 (exact power-of-2, so the bf16
    # path is bit-identical after the host-side /64 of the output) to put
    # the fp8 copies of k-tiles 0-1 in e4m3's normal range.
    w64 = np.asarray(expert_w, dtype=np.float32) * np.float32(64.0)
    w_bf = w64.astype(ml_dtypes.bfloat16)
    # w8[e, p, j, o] = e4m3(64*w[e, j*128+p, o]) for k-tiles j=0,1
    w8 = np.ascontiguousarray(
        w64[:, : 2 * P, :]
        .reshape(_E, 2, P, _DOUT)
        .transpose(0, 2, 1, 3)
    ).astype(ml_dtypes.float8_e4m3)
    gwt_bf = _host_gwt(np.asarray(gate_w)).astype(ml_dtypes.bfloat16)
    expb = np.broadcast_to(
        np.exp(np.asarray(gate_b, dtype=np.float64)).astype(np.float32),
        (P, _E),
    ).copy()

    in_maps = []
    for c in range(N_CORES):
        shard = tokens[c * tpc:(c + 1) * tpc]  # [1024, 1024]
        xT32 = np.ascontiguousarray(shard.T)   # [1024(D), 1024(T)] f32
        xT = xT32.astype(ml_dtypes.bfloat16)
        # DoubleRowSwInterleave stationary layout (see build_moe_nc):
        # x8T[p, g, 2*(127-u)+j] = e4m3(x[g*128+u, j*128+p])
        mt = tpc // P
        a = xT32[: 2 * P].reshape(2, P, mt, P)        # [j, p, g, u]
        x8T = np.ascontiguousarray(
            a[:, :, :, ::-1].transpose(1, 2, 3, 0)    # [p, g, 127-u, j]
        ).reshape(P, mt, 2 * P).astype(ml_dtypes.float8_e4m3)
        in_maps.append(
            {"xT": xT, "x8T": x8T, "w": w_bf, "w8": w8,
             "gwt": gwt_bf, "expb": expb}
        )

    nc = build_moe_nc(T=tpc, D=_DIN, O=_DOUT, E=_E)
    res = run_bass_kernel_spmd(nc, in_maps, list(range(N_CORES)), trace=_trace)
    LAST_RESULTS = res
    # reassemble the flat tile-major output (must mirror the kernel's
    # tile map: n0 full-width per m; final block halves, m7 quarters)
    tile_map = [(m, 0, 512) for m in range(8)]
    for m in range(7):
        tile_map += [(m, 512, 256), (m, 768, 256)]
    tile_map += [(7, 512 + q * 128, 128) for q in range(4)]
    tpc_o = tpc * _DOUT
    parts = []
    for c in range(N_CORES):
        flat = np.asarray(res.results[c]["out"]).reshape(tpc_o)
        full_c = np.empty((tpc, _DOUT), dtype=np.float32)
        for (m, c0, ncol) in tile_map:
            off = m * P * _DOUT + c0 * P
            full_c[m * P:(m + 1) * P, c0:c0 + ncol] = flat[
                off:off + P * ncol
            ].reshape(P, ncol)
        parts.append(full_c)
    full = np.concatenate(parts, axis=0).astype(np.float32)
    full *= np.float32(1.0 / 64.0)  # undo the x64 weight scaling (exact)
    return full.reshape(_B, _S, _DOUT)

